# revision 1
# baseline (speedup 1.0000x reference)
"""Trainium2 Bass kernel: 8-head attention block (BN-folded projections,
relative-position bias, softmax, GELU + output projection).

Sharding: data-parallel over batch across 8 NeuronCores (2 batch elems/core).
All weights / bias tables replicated; no collectives.

Per-core layout strategy (all "T" tensors are [channel, position]):
  XT  [c=256, n=1024]  bf16   (host pre-transposed, pre-cast)
  QT/KT [d=256, n]     bf16   rows grouped 32/head -> 4 heads per 128-row tile
  V    [n, 512]        bf16   (natural layout, heads contiguous 64-wide)
  dotsT[j, i] psum f32 = sum_d KT[d,j] QT[d,i]  (4 heads packed via row groups)
                       + bias[j,i]/scale^2      (identity-matmul accumulate)
  exp  = ScalarE Exp(scale * psum) -> bf16
  AV: outT[d, i] = sum_j V[j,d] exp[j,i], head pairs packed via PE col groups
      so pair output fills psum partitions 0..127 = gelu tile layout.
  softmax sums via 64-wide all-ones stationary matmuls in the same col
  groups as AV, so each head's sum is replicated across the same 64
  partitions as its AV rows (no cross-partition moves); reciprocal on DVE,
  one fused mul -> gelu input.
  GELU (ScalarE, +BN_v offset as per-partition bias) -> out proj -> +bias -> DMA.

  HW workarounds (device crashes otherwise, found by probing):
  - tile_position (96,0) is fatal (quadrant-3 bug): head 3 runs as a K=64
    matmul at (64,0) against a KT copy with head-2 rows zeroed.
  - tile_position'd matmuls need PSUM-bank-aligned outputs: every packed
    matmul output is a full 2KB bank.
  - exp and gelu are forced into disjoint program phases so the ScalarE
    activation table loads exactly twice.
"""

import os
import numpy as np
import ml_dtypes

import concourse.bass as bass
import concourse.tile as tile
from concourse import bacc, mybir
from concourse.bass_utils import run_bass_kernel_spmd
from concourse.tile import add_dep_helper

NPBF16 = ml_dtypes.bfloat16
BF16 = mybir.dt.bfloat16
F32 = mybir.dt.float32

HEADS, DK, DV = 8, 32, 64
N = 1024          # positions = 32*32
C = 256           # channels
IDV = HEADS * DV  # 512
NCORES = 8
BLOC = 2          # batch elems per core
SCALE = float(DK) ** -0.5
EPS = 1e-5

_CACHE = {}


def _build_nc():
    nc = bacc.Bacc("TRN2", target_bir_lowering=False, debug=False)

    xt_d = nc.declare_dram_parameter("xt", [BLOC, 2, 128, N], BF16, isOutput=False)
    wq_d = nc.declare_dram_parameter("wq", [128, 2, C], BF16, isOutput=False)
    wk_d = nc.declare_dram_parameter("wk", [128, 2, C], BF16, isOutput=False)
    wv_d = nc.declare_dram_parameter("wv", [128, 2, IDV], BF16, isOutput=False)
    wo_d = nc.declare_dram_parameter("wo", [128, 4, C], BF16, isOutput=False)
    oq_d = nc.declare_dram_parameter("oq", [128, 2], F32, isOutput=False)
    ok_d = nc.declare_dram_parameter("ok", [128, 2], F32, isOutput=False)
    ovg_d = nc.declare_dram_parameter("ovg", [128, 4], F32, isOutput=False)
    bout_d = nc.declare_dram_parameter("bout", [128, C], F32, isOutput=False)
    # bias[hg, is, jt, j1, h, i1] = pos_bias[j, i, 4*hg+h] / SCALE^2
    bias_d = nc.declare_dram_parameter("bias", [2, 2, 8, 128, 4, 512], BF16,
                                       isOutput=False)
    id_d = nc.declare_dram_parameter("ident", [128, 128], BF16, isOutput=False)
    out_d = nc.declare_dram_parameter("out", [BLOC, N, C], F32, isOutput=True)

    Exp = mybir.ActivationFunctionType.Exp
    Gelu = mybir.ActivationFunctionType.Gelu

    with tile.TileContext(nc) as tc:
        with (
            tc.tile_pool(name="const", bufs=1) as const,
            tc.tile_pool(name="persist", bufs=1) as persist,
            tc.tile_pool(name="biasp", bufs=12) as biasp,
            tc.tile_pool(name="expp", bufs=10) as expp,
            tc.tile_pool(name="recp", bufs=3) as recp,
            tc.tile_pool(name="outp", bufs=4) as outp,
            tc.tile_pool(name="dpsum", bufs=3, space="PSUM") as dpsum,
            tc.tile_pool(name="avpsum", bufs=1, space="PSUM") as avpsum,
        ):
            dma = nc.sync

            # ---------------- constants ----------------
            wq_s = const.tile([128, 2, C], BF16, tag="wq")
            dma.dma_start(wq_s[:], wq_d[:])
            wk_s = const.tile([128, 2, C], BF16, tag="wk")
            dma.dma_start(wk_s[:], wk_d[:])
            wv_s = const.tile([128, 2, IDV], BF16, tag="wv")
            dma.dma_start(wv_s[:], wv_d[:])
            wo_s = const.tile([128, 4, C], BF16, tag="wo")
            dma.dma_start(wo_s[:], wo_d[:])
            oq_s = const.tile([128, 2], F32, tag="oq")
            dma.dma_start(oq_s[:], oq_d[:])
            ok_s = const.tile([128, 2], F32, tag="ok")
            dma.dma_start(ok_s[:], ok_d[:])
            ovg_s = const.tile([128, 4], F32, tag="ovg")
            dma.dma_start(ovg_s[:], ovg_d[:])
            bout_s = const.tile([128, C], F32, tag="bout")
            dma.dma_start(bout_s[:], bout_d[:])
            ident_s = const.tile([128, 128], BF16, tag="ident")
            dma.dma_start(ident_s[:], id_d[:])
            ones_s = const.tile([128, 64], BF16, tag="ones")
            nc.vector.memset(ones_s[:], 1.0)

            # ---------------- load x (pre-transposed on host) ----------------
            xt = {}
            for b in range(BLOC):
                for ct in range(2):
                    t = persist.tile([128, N], BF16, tag=f"xt{b}{ct}", name=f"xt{b}{ct}")
                    dma.dma_start(t[:], xt_d[b, ct])
                    xt[b, ct] = t

            # ---------------- Q/K projections -> QT/KT [d, i] bf16 ----------
            qt, kt, kzt = {}, {}, {}
            for b in range(BLOC):
                for tt in range(2):  # 128-row d-tile == head group tt
                    qtile = persist.tile([128, N], BF16, tag=f"qt{b}{tt}", name=f"qt{b}{tt}")
                    ktile = persist.tile([128, N], BF16, tag=f"kt{b}{tt}", name=f"kt{b}{tt}")
                    qt[b, tt], kt[b, tt] = qtile, ktile
                    # kz: copy of KT with head-2 rows zeroed; lets head 3 run
                    # as a K=64 matmul at row group 2 (tile_position (96,0)
                    # crashes this runtime: quadrant-3 HW bug)
                    kz = persist.tile([128, N], BF16, tag=f"kz{b}{tt}",
                                      name=f"kz{b}{tt}")
                    kzt[b, tt] = kz
                    nc.vector.memset(kz[64:96, :], 0.0)
                    for wsb, osb, dst in ((wq_s, oq_s, qtile), (wk_s, ok_s, ktile)):
                        for i2 in range(2):
                            ps = dpsum.tile([128, 512], F32, tag="dps")
                            for ct in range(2):
                                nc.tensor.matmul(
                                    ps[:],
                                    wsb[:, ct, tt * 128:(tt + 1) * 128],
                                    xt[b, ct][:, i2 * 512:(i2 + 1) * 512],
                                    start=(ct == 0), stop=(ct == 1),
                                )
                            nc.vector.tensor_scalar_add(
                                dst[:, i2 * 512:(i2 + 1) * 512], ps[:],
                                osb[:, tt:tt + 1])
                            if dst is ktile:
                                nc.vector.tensor_scalar_add(
                                    kz[96:128, i2 * 512:(i2 + 1) * 512],
                                    ps[96:128, :], osb[96:128, tt:tt + 1])

            # ---------------- V projection -> V [j, (h d)] bf16 --------------
            vt = {}
            for b in range(BLOC):
                for it in range(8):
                    v = persist.tile([128, 8, DV], BF16, tag=f"v{b}{it}", name=f"v{b}{it}")
                    vt[b, it] = v
                    ps = dpsum.tile([128, 512], F32, tag="dps")
                    for ct in range(2):
                        nc.tensor.matmul(
                            ps[:],
                            xt[b, ct][:, it * 128:(it + 1) * 128],
                            wv_s[:, ct, :],
                            start=(ct == 0), stop=(ct == 1),
                        )
                    nc.vector.tensor_copy(v[:, :, :], ps[:].rearrange("p (h d) -> p h d", h=8))

            # gelu input tiles [128 = head pair, 1024] bf16, per (b, dt)
            gelu_t = {}
            for b in range(BLOC):
                for dt in range(4):
                    gelu_t[b, dt] = persist.tile([128, N], BF16, tag=f"g{b}{dt}", name=f"g{b}{dt}")

            last_exp = [None]
            # ---------------- attention ----------------
            for hg in range(2):          # head group of 4 (row-packed dots)
                for isl in range(2):     # i slice of 512
                    bias_t = {}
                    for jt in range(8):
                        bt = biasp.tile([128, 4, 512], BF16, tag="bias", name=f"bias{hg}{isl}{jt}")
                        dma.dma_start(bt[:], bias_d[hg, isl, jt])
                        bias_t[jt] = bt
                    for b in range(BLOC):
                        # --- dots + bias + exp over all j tiles ---
                        # tile_position'd matmul outputs must be PSUM
                        # bank-aligned (half-bank offsets crash the device),
                        # so each head gets a full 512-wide bank.
                        exp_t = {}
                        i0 = isl * 512
                        for jt in range(8):
                            et = expp.tile([128, 4, 512], BF16, tag="exp", name=f"exp{b}{jt}")
                            exp_t[jt] = et
                            dtiles = []
                            for pair in range(2):
                                dps = dpsum.tile([128, 2, 512], F32, tag="dps")
                                dtiles.append(dps)
                                for half in range(2):
                                    h = 2 * pair + half
                                    if h < 3:
                                        nc.tensor.matmul(
                                            dps[:, half, :],
                                            kt[b, hg][32 * h:32 * h + 32,
                                                      jt * 128:(jt + 1) * 128],
                                            qt[b, hg][32 * h:32 * h + 32,
                                                      i0:i0 + 512],
                                            start=True, stop=False,
                                            tile_position=(32 * h, 0),
                                        )
                                    else:
                                        # head 3: K=64 at row group 2 with
                                        # head-2 weight rows zeroed
                                        # (tile_position (96,0) is broken)
                                        nc.tensor.matmul(
                                            dps[:, half, :],
                                            kzt[b, hg][64:128,
                                                       jt * 128:(jt + 1) * 128],
                                            qt[b, hg][64:128, i0:i0 + 512],
                                            start=True, stop=False,
                                            tile_position=(64, 0),
                                        )
                            # all bias matmuls together: identity stationary
                            # loads once per j tile
                            for pair in range(2):
                                for half in range(2):
                                    h = 2 * pair + half
                                    nc.tensor.matmul(
                                        dtiles[pair][:, half, :],
                                        ident_s[:],
                                        bias_t[jt][:, h, :],
                                        start=False, stop=True,
                                    )
                            for pair in range(2):
                                ae = nc.scalar.activation(
                                    et[:, 2 * pair:2 * pair + 2, :],
                                    dtiles[pair][:], Exp, scale=SCALE)
                                last_exp[0] = ae
                        # --- AV + softmax sums, head pairs in col groups ---
                        for p in range(2):
                            av = avpsum.tile([128, 512], F32, tag="av")
                            sums = avpsum.tile([128, 512], F32, tag="sums")
                            for half in range(2):
                                h = 2 * p + half
                                hglob = 4 * hg + h
                                for jt in range(8):
                                    nc.tensor.matmul(
                                        av[64 * half:64 * half + 64, :],
                                        vt[b, jt][:, hglob, :],
                                        exp_t[jt][:, h, :],
                                        start=(jt == 0), stop=(jt == 7),
                                        tile_position=(0, 64 * half),
                                    )
                            # ones-stationary sums after av: ones load once,
                            # replicated sum lands on the same partitions as av
                            for half in range(2):
                                h = 2 * p + half
                                for jt in range(8):
                                    nc.tensor.matmul(
                                        sums[64 * half:64 * half + 64, :],
                                        ones_s[:],
                                        exp_t[jt][:, h, :],
                                        start=(jt == 0), stop=(jt == 7),
                                        tile_position=(0, 64 * half),
                                    )
                            rec = recp.tile([128, 512], F32, tag="rec")
                            nc.vector.reciprocal(rec[:], sums[:])
                            dt = 2 * hg + p
                            nc.vector.tensor_mul(
                                gelu_t[b, dt][:, isl * 512:(isl + 1) * 512],
                                av[:], rec[:])

            # ---------------- GELU + output projection ----------------
            for b in range(BLOC):
                for dt in range(4):
                    gi = nc.scalar.activation(gelu_t[b, dt][:], gelu_t[b, dt][:],
                                              Gelu, bias=ovg_s[:, dt:dt + 1],
                                              scale=1.0)
                    if last_exp[0] is not None:
                        add_dep_helper(gi.ins, last_exp[0].ins, sync=False,
                                       reason="group ACT table sets")
            for b in range(BLOC):
                for it in range(8):
                    ops = avpsum.tile([128, C], F32, tag="sums")
                    for dt in range(4):
                        nc.tensor.matmul(
                            ops[:],
                            gelu_t[b, dt][:, it * 128:(it + 1) * 128],
                            wo_s[:, dt, :],
                            start=(dt == 0), stop=(dt == 3),
                        )
                    osb = outp.tile([128, C], F32, tag="osb")
                    nc.vector.tensor_add(osb[:], ops[:], bout_s[:])
                    dma.dma_start(out_d[b, it * 128:(it + 1) * 128, :], osb[:])

    nc.compile()
    return nc


def _host_prep(x, w_q, bn_q, w_k, bn_k, w_v, bn_v, w_out, b_out, bn_out,
               pos_table):
    """Fold BN into weights, build bias table, shard across cores."""
    def fold(bn):
        g, b_, m, v = [np.asarray(a, np.float64) for a in bn]
        s = g / np.sqrt(v + EPS)
        return s, b_ - m * s

    sq, oq = fold(bn_q)
    sk, ok = fold(bn_k)
    sv, ov = fold(bn_v)
    so, oo = fold(bn_out)

    def wtile(w, s, ncols):
        # [C_in, D] * s[D] -> [128, C_in//128, D] bf16 (partition-major)
        w_eff = (np.asarray(w, np.float64) * s[None, :]).astype(np.float32)
        return np.ascontiguousarray(
            w_eff.reshape(-1, 128, ncols).transpose(1, 0, 2)).astype(NPBF16)

    wq = wtile(w_q, sq, C)
    wk = wtile(w_k, sk, C)
    wv = wtile(w_v, sv, IDV)
    wo = wtile(w_out, so, C)

    oq_t = np.ascontiguousarray(oq.astype(np.float32).reshape(2, 128).T)
    ok_t = np.ascontiguousarray(ok.astype(np.float32).reshape(2, 128).T)
    ovg_t = np.ascontiguousarray(ov.astype(np.float32).reshape(4, 128).T)
    bout_eff = (np.asarray(b_out, np.float64) * so + oo).astype(np.float32)
    bout_t = np.ascontiguousarray(np.broadcast_to(bout_eff, (128, C)))

    # position bias table
    r = np.arange(32)
    pos = np.stack(np.meshgrid(r, r, indexing="ij"), axis=-1).reshape(-1, 2)
    rel = np.abs(pos[:, None, :] - pos[None, :, :])
    idx = rel[..., 0] * 32 + rel[..., 1]           # [n, n]
    bias = np.asarray(pos_table, np.float32)[idx]  # [j, i, 8]
    bias = bias / (SCALE * SCALE)
    # -> [hg, is, jt, j1, h, i1]
    bias = bias.reshape(8, 128, 2, 512, 2, 4)      # jt, j1, is, i1, hg, h
    bias = np.ascontiguousarray(
        bias.transpose(4, 2, 0, 1, 5, 3)).astype(NPBF16)

    ident = np.eye(128, dtype=NPBF16)

    x = np.asarray(x, np.float32).reshape(-1, N, C)      # [B, n, C]
    common = dict(wq=wq, wk=wk, wv=wv, wo=wo, oq=oq_t, ok=ok_t, ovg=ovg_t,
                  bout=bout_t, bias=bias, ident=ident)
    in_maps = []
    for c in range(NCORES):
        xl = x[c * BLOC:(c + 1) * BLOC]                  # [2, n, C]
        xtl = xl.transpose(0, 2, 1).reshape(BLOC, 2, 128, N).astype(NPBF16)
        in_maps.append(dict(common, xt=np.ascontiguousarray(xtl)))
    return in_maps


def kernel(**inputs):
    if "nc" not in _CACHE:
        _CACHE["nc"] = _build_nc()
    nc = _CACHE["nc"]
    in_maps = _host_prep(**inputs)
    res = run_bass_kernel_spmd(nc, in_maps, core_ids=list(range(NCORES)),
                               trace=bool(int(os.environ.get("KTRACE", "0"))))
    _CACHE["last_result"] = res
    outs = [res.results[c]["out"].reshape(BLOC, 32, 32, C)
            for c in range(NCORES)]
    return np.concatenate(outs, axis=0).astype(np.float32)


if __name__ == "__main__":
    nc = _build_nc()
    print("build + compile OK")



# revision 2
# speedup vs baseline: 1.2261x; 1.2261x over previous
"""Trainium2 Bass kernel: 8-head attention block (BN-folded projections,
relative-position bias, softmax, GELU + output projection).

Sharding: data-parallel over batch across 8 NeuronCores (2 batch elems/core).
All weights / bias tables replicated; no collectives.

Engine-balanced design (TimelineSim cost model charges matmuls by OUTPUT
free-size only, ldweights free, no credit for tile_position packing):

  ACT is the irreducible bottleneck: exp of the 2*8*1024*1024 attention
  logits = 131072 free-elems (~109us) + per-instr init. Everything else is
  kept off ACT:
  - relative-position bias applied as a POST-EXP multiply by the
    precomputed table E = exp(bias/scale) on DVE (bf16 2x mode), instead
    of identity-matmul accumulation on PE.
  - softmax denominators come free from AV: the AV moving operand is
    V augmented with a ones column ([j, 65]), AV computed in [i, d]
    orientation (out free = 65 per matmul instead of 512), so AV+sums
    cost 66560 PE cycles instead of 262144.
  - AV's [i, hd] output is PE-transposed (128x128 tiles) back to [hd, i]
    for the GELU + output projection; BN_v offset rides the GELU bias.

  Per-core engine totals (cost model): PE ~107us, ACT ~144us (bound),
  DVE ~120us, DMA ~58us.

  HW workaround kept from the earlier kernel: tile_position (96,0) is
  fatal (quadrant-3 bug), so heads 3/7's dots run as K=64 matmuls at
  (64,0) against a KT copy with the neighbouring head's rows zeroed.
"""

import os
import numpy as np
import ml_dtypes

import concourse.bass as bass
import concourse.tile as tile
from concourse import bacc, mybir
from concourse.bass_utils import run_bass_kernel_spmd

NPBF16 = ml_dtypes.bfloat16
BF16 = mybir.dt.bfloat16
F32 = mybir.dt.float32

HEADS, DK, DV = 8, 32, 64
N = 1024          # positions = 32*32
C = 256           # channels
IDV = HEADS * DV  # 512
NCORES = 8
BLOC = 2          # batch elems per core
SCALE = float(DK) ** -0.5
EPS = 1e-5

_CACHE = {}


def _build_nc():
    nc = bacc.Bacc("TRN2", target_bir_lowering=False, debug=False)

    xt_d = nc.declare_dram_parameter("xt", [BLOC, 2, 128, N], BF16, isOutput=False)
    wq_d = nc.declare_dram_parameter("wq", [128, 2, C], BF16, isOutput=False)
    wk_d = nc.declare_dram_parameter("wk", [128, 2, C], BF16, isOutput=False)
    wv_d = nc.declare_dram_parameter("wv", [128, 2, IDV], BF16, isOutput=False)
    wo_d = nc.declare_dram_parameter("wo", [128, 4, C], BF16, isOutput=False)
    oq_d = nc.declare_dram_parameter("oq", [128, 2], F32, isOutput=False)
    ok_d = nc.declare_dram_parameter("ok", [128, 2], F32, isOutput=False)
    ovg_d = nc.declare_dram_parameter("ovg", [128, 4], F32, isOutput=False)
    bout_d = nc.declare_dram_parameter("bout", [128, C], F32, isOutput=False)
    # E[h, jt, j1, i] = exp(pos_bias[j, i, h] / SCALE)
    e_d = nc.declare_dram_parameter("etab", [HEADS, 8, 128, N], BF16,
                                    isOutput=False)
    id_d = nc.declare_dram_parameter("ident", [128, 128], BF16, isOutput=False)
    out_d = nc.declare_dram_parameter("out", [BLOC, N, C], F32, isOutput=True)

    Exp = mybir.ActivationFunctionType.Exp
    Gelu = mybir.ActivationFunctionType.Gelu

    with tile.TileContext(nc) as tc:
        with (
            tc.tile_pool(name="const", bufs=1) as const,
            tc.tile_pool(name="persist", bufs=1) as persist,
            tc.tile_pool(name="ep", bufs=6) as ep,        # E tiles in flight
            tc.tile_pool(name="etp", bufs=4) as etp,      # raw exp tiles
            tc.tile_pool(name="et2p", bufs=36) as et2p,   # attn-weight tiles
            tc.tile_pool(name="recp", bufs=4) as recp,
            tc.tile_pool(name="outp", bufs=4) as outp,
            tc.tile_pool(name="dpsum", bufs=2, space="PSUM") as dpsum,
            tc.tile_pool(name="avpsum", bufs=1, space="PSUM") as avpsum,
            tc.tile_pool(name="tpsum", bufs=2, space="PSUM") as tpsum,
            tc.tile_pool(name="opsum", bufs=1, space="PSUM") as opsum,
        ):
            dma = nc.sync

            # ---------------- constants ----------------
            wq_s = const.tile([128, 2, C], BF16, tag="wq")
            dma.dma_start(wq_s[:], wq_d[:])
            wk_s = const.tile([128, 2, C], BF16, tag="wk")
            dma.dma_start(wk_s[:], wk_d[:])
            wv_s = const.tile([128, 2, IDV], BF16, tag="wv")
            dma.dma_start(wv_s[:], wv_d[:])
            wo_s = const.tile([128, 4, C], BF16, tag="wo")
            dma.dma_start(wo_s[:], wo_d[:])
            oq_s = const.tile([128, 2], F32, tag="oq")
            dma.dma_start(oq_s[:], oq_d[:])
            ok_s = const.tile([128, 2], F32, tag="ok")
            dma.dma_start(ok_s[:], ok_d[:])
            ovg_s = const.tile([128, 4], F32, tag="ovg")
            dma.dma_start(ovg_s[:], ovg_d[:])
            bout_s = const.tile([128, C], F32, tag="bout")
            dma.dma_start(bout_s[:], bout_d[:])
            ident_s = const.tile([128, 128], BF16, tag="ident")
            dma.dma_start(ident_s[:], id_d[:])

            # ---------------- load x (pre-transposed on host) ----------------
            xt = {}
            for b in range(BLOC):
                for ct in range(2):
                    t = persist.tile([128, N], BF16, tag=f"xt{b}{ct}",
                                     name=f"xt{b}{ct}")
                    dma.dma_start(t[:], xt_d[b, ct])
                    xt[b, ct] = t

            # ---------------- Q/K projections -> QT/KT [d, i] bf16 ----------
            qt, kt, kzt = {}, {}, {}
            for b in range(BLOC):
                for dt in range(2):
                    qtile = persist.tile([128, N], BF16, tag=f"qt{b}{dt}",
                                         name=f"qt{b}{dt}")
                    ktile = persist.tile([128, N], BF16, tag=f"kt{b}{dt}",
                                         name=f"kt{b}{dt}")
                    qt[b, dt], kt[b, dt] = qtile, ktile
                    # kz: KT copy with head-2 rows zeroed so head 3 can run
                    # as K=64 at tile_position (64,0) ((96,0) crashes).
                    kz = persist.tile([128, N], BF16, tag=f"kz{b}{dt}",
                                      name=f"kz{b}{dt}")
                    kzt[b, dt] = kz
                    nc.gpsimd.memset(kz[64:96, :], 0.0)
                    for wsb, osb, dst in ((wq_s, oq_s, qtile), (wk_s, ok_s, ktile)):
                        ps = dpsum.tile([128, 2, 512], F32, tag="dps")
                        for ih in range(2):
                            for ct in range(2):
                                nc.tensor.matmul(
                                    ps[:, ih, :],
                                    wsb[:, ct, dt * 128:(dt + 1) * 128],
                                    xt[b, ct][:, ih * 512:(ih + 1) * 512],
                                    start=(ct == 0), stop=(ct == 1),
                                )
                        nc.vector.tensor_scalar_add(
                            dst[:].rearrange("p (a f) -> p a f", a=2),
                            ps[:], osb[:, dt:dt + 1])
                        if dst is ktile:
                            nc.vector.tensor_scalar_add(
                                kz[96:128, :].rearrange("p (a f) -> p a f", a=2),
                                ps[96:128, :, :], osb[96:128, dt:dt + 1])

            # ------------- V projection -> Vaug [j, (h, 65)] bf16 ------------
            # column 64 of every head group preset to 1.0 -> softmax sums
            # come out of the AV matmul for free.
            vt = {}
            for b in range(BLOC):
                for jt in range(8):
                    v = persist.tile([128, HEADS, DV + 1], BF16, tag=f"v{b}{jt}",
                                     name=f"v{b}{jt}")
                    vt[b, jt] = v
                    nc.gpsimd.memset(v[:, :, 64:65], 1.0)
                    ps = dpsum.tile([128, 2, 512], F32, tag="dps")
                    for ct in range(2):
                        nc.tensor.matmul(
                            ps[:, 0, :],
                            xt[b, ct][:, jt * 128:(jt + 1) * 128],
                            wv_s[:, ct, :],
                            start=(ct == 0), stop=(ct == 1),
                        )
                    nc.vector.tensor_copy(
                        v[:, :, 0:64],
                        ps[:, 0, :].rearrange("p (h d) -> p h d", h=HEADS))

            # gp[b, it]: normalized attention output, [i, (h, d)] layout
            gp = {}
            for b in range(BLOC):
                for it in range(8):
                    gp[b, it] = persist.tile([128, HEADS, DV], BF16,
                                             tag=f"gp{b}{it}", name=f"gp{b}{it}")

            # ---------------- attention ----------------
            for hp in range(4):              # head pairs
                et2 = {}
                for hl in range(2):
                    h = 2 * hp + hl
                    dt, hq = h // 4, h % 4
                    for jt in range(8):
                        etab = ep.tile([128, N], BF16, tag="etab",
                                       name=f"e{h}{jt}")
                        dma.dma_start(etab[:], e_d[h, jt])
                        for b in range(BLOC):
                            dps = dpsum.tile([128, 2, 512], F32, tag="dps")
                            for ih in range(2):
                                if hq < 3:
                                    nc.tensor.matmul(
                                        dps[:, ih, :],
                                        kt[b, dt][32 * hq:32 * hq + 32,
                                                  jt * 128:(jt + 1) * 128],
                                        qt[b, dt][32 * hq:32 * hq + 32,
                                                  ih * 512:(ih + 1) * 512],
                                        start=True, stop=True,
                                        tile_position=(32 * hq, 0),
                                    )
                                else:
                                    nc.tensor.matmul(
                                        dps[:, ih, :],
                                        kzt[b, dt][64:128,
                                                   jt * 128:(jt + 1) * 128],
                                        qt[b, dt][64:128,
                                                  ih * 512:(ih + 1) * 512],
                                        start=True, stop=True,
                                        tile_position=(64, 0),
                                    )
                            et = etp.tile([128, 2, 512], BF16, tag="et")
                            nc.scalar.activation(et[:], dps[:], Exp, scale=SCALE)
                            t2 = et2p.tile([128, N], BF16, tag="et2",
                                           name=f"et2_{h}{jt}{b}")
                            et2[hl, jt, b] = t2
                            nc.vector.tensor_tensor(
                                t2[:].rearrange("p (a f) -> p a f", a=2),
                                et[:], etab[:].rearrange("p (a f) -> p a f", a=2),
                                mybir.AluOpType.mult)

                # ---- AV + sums (merged), [i, d] orientation ----
                for b in range(BLOC):
                    for it in range(8):
                        av = avpsum.tile([128, 2, DV + 1], F32, tag="av")
                        for hl in range(2):
                            h = 2 * hp + hl
                            for jt in range(8):
                                nc.tensor.matmul(
                                    av[:, hl, :],
                                    et2[hl, jt, b][:, it * 128:(it + 1) * 128],
                                    vt[b, jt][:, h, :],
                                    start=(jt == 0), stop=(jt == 7),
                                )
                        rec = recp.tile([128, 2], F32, tag="rec")
                        nc.vector.reciprocal(rec[:], av[:, :, 64])
                        nc.vector.tensor_tensor(
                            gp[b, it][:, 2 * hp:2 * hp + 2, :],
                            av[:, :, 0:64],
                            rec[:, :, None].broadcast_to((128, 2, DV)),
                            mybir.AluOpType.mult)

            # -------- transpose [i, hd] -> [hd, i], GELU, out proj ----------
            gelu_t = {}
            for b in range(BLOC):
                for dt in range(4):
                    tps = tpsum.tile([128, 8, 128], BF16, tag="tps")
                    for it in range(8):
                        nc.tensor.transpose(
                            tps[:, it, :],
                            gp[b, it][:, 2 * dt:2 * dt + 2, :],
                            ident_s[:])
                    g = persist.tile([128, N], BF16, tag=f"g{b}{dt}",
                                     name=f"g{b}{dt}")
                    gelu_t[b, dt] = g
                    nc.scalar.activation(g[:].rearrange("p (a f) -> p a f", a=8),
                                         tps[:], Gelu,
                                         bias=ovg_s[:, dt:dt + 1], scale=1.0)

            for b in range(BLOC):
                for it in range(8):
                    ops = opsum.tile([128, C], F32, tag="ops")
                    for dt in range(4):
                        nc.tensor.matmul(
                            ops[:],
                            gelu_t[b, dt][:, it * 128:(it + 1) * 128],
                            wo_s[:, dt, :],
                            start=(dt == 0), stop=(dt == 3),
                        )
                    osb = outp.tile([128, C], F32, tag="osb")
                    nc.vector.tensor_tensor(osb[:], ops[:], bout_s[:],
                                            mybir.AluOpType.add)
                    dma.dma_start(out_d[b, it * 128:(it + 1) * 128, :], osb[:])

    nc.compile()
    return nc


def _host_prep(x, w_q, bn_q, w_k, bn_k, w_v, bn_v, w_out, b_out, bn_out,
               pos_table):
    """Fold BN into weights, build exp-bias table, shard across cores."""
    def fold(bn):
        g, b_, m, v = [np.asarray(a, np.float64) for a in bn]
        s = g / np.sqrt(v + EPS)
        return s, b_ - m * s

    sq, oq = fold(bn_q)
    sk, ok = fold(bn_k)
    sv, ov = fold(bn_v)
    so, oo = fold(bn_out)

    def wtile(w, s, ncols):
        w_eff = (np.asarray(w, np.float64) * s[None, :]).astype(np.float32)
        return np.ascontiguousarray(
            w_eff.reshape(-1, 128, ncols).transpose(1, 0, 2)).astype(NPBF16)

    wq = wtile(w_q, sq, C)
    wk = wtile(w_k, sk, C)
    wv = wtile(w_v, sv, IDV)
    wo = wtile(w_out, so, C)

    oq_t = np.ascontiguousarray(oq.astype(np.float32).reshape(2, 128).T)
    ok_t = np.ascontiguousarray(ok.astype(np.float32).reshape(2, 128).T)
    ovg_t = np.ascontiguousarray(ov.astype(np.float32).reshape(4, 128).T)
    bout_eff = (np.asarray(b_out, np.float64) * so + oo).astype(np.float32)
    bout_t = np.ascontiguousarray(np.broadcast_to(bout_eff, (128, C)))

    # E[h, jt, j1, i] = exp(bias[j, i, h] / SCALE)
    r = np.arange(32)
    pos = np.stack(np.meshgrid(r, r, indexing="ij"), axis=-1).reshape(-1, 2)
    rel = np.abs(pos[:, None, :] - pos[None, :, :])
    idx = rel[..., 0] * 32 + rel[..., 1]                 # [j, i]
    bias = np.asarray(pos_table, np.float32)[idx]        # [j, i, 8]
    etab = np.exp(bias / SCALE).transpose(2, 0, 1)       # [8, j, i]
    etab = np.ascontiguousarray(
        etab.reshape(HEADS, 8, 128, N)).astype(NPBF16)

    ident = np.eye(128, dtype=NPBF16)

    x = np.asarray(x, np.float32).reshape(-1, N, C)      # [B, n, C]
    common = dict(wq=wq, wk=wk, wv=wv, wo=wo, oq=oq_t, ok=ok_t, ovg=ovg_t,
                  bout=bout_t, etab=etab, ident=ident)
    in_maps = []
    for c in range(NCORES):
        xl = x[c * BLOC:(c + 1) * BLOC]                  # [2, n, C]
        xtl = xl.transpose(0, 2, 1).reshape(BLOC, 2, 128, N).astype(NPBF16)
        in_maps.append(dict(common, xt=np.ascontiguousarray(xtl)))
    return in_maps


def kernel(**inputs):
    if "nc" not in _CACHE:
        _CACHE["nc"] = _build_nc()
    nc = _CACHE["nc"]
    in_maps = _host_prep(**inputs)
    res = run_bass_kernel_spmd(nc, in_maps, core_ids=list(range(NCORES)),
                               trace=bool(int(os.environ.get("KTRACE", "0"))))
    _CACHE["last_result"] = res
    outs = [res.results[c]["out"].reshape(BLOC, 32, 32, C)
            for c in range(NCORES)]
    return np.concatenate(outs, axis=0).astype(np.float32)


if __name__ == "__main__":
    nc = _build_nc()
    print("build + compile OK")


# revision 26
# speedup vs baseline: 1.6879x; 1.3766x over previous
"""Trainium2 Bass kernel: 8-head attention block (BN-folded projections,
relative-position bias, softmax, GELU + output projection).

Sharding: data-parallel over batch across 8 NeuronCores (2 batch elems/core).
All weights / bias tables replicated; no collectives.

Engine-balanced design for the TimelineSim cost model (matmuls charge
output free-size on PE.ENGINE plus ~149ns fixed on PE.SEQ per
matmul+ldweights pair; ldweights engine time is free; no credit for
tile_position packing; engines execute their instruction streams
in order):

  ACT is the irreducible bottleneck: exp of the 2*8*1024*1024 attention
  logits = 131072 free-elems (~109us) + per-instr init + gelu ~= 144us
  of ACT busy. Everything else is kept under that and pipelined so ACT
  never starves:
  - relative-position bias applied as a POST-EXP multiply by the
    precomputed table E = exp(bias/scale) on DVE (bf16 2x mode; ~1/4 of
    tiles on GpSimd), instead of identity-matmul accumulation on PE.
  - softmax denominators come free from AV: stationary is V augmented
    with 64 ones columns ([j, 128] = V_h | ones), so one matmul stream
    yields AV on partitions 0:64 and the softmax sums on 64:128.
    AV+sums: 256 matmuls / 131072 PE cycles.
  - normalize uses partition-shifted DVE ops (recip rows 64:128 ->
    rows 0:64, multiply into the packed gelu-layout tile).
  - software pipelining: head h's AV matmuls are emitted inside head
    h+1's dots/exp loop (PE stream has slack there); the V projection
    and the second Q/K projection tile ride the h0/h1 loops the same
    way; x is DMA'd before the weights.
  - GELUs are dep-forced after all exps so only 2 activation-table
    loads are emitted.

  HW workaround: tile_position (96,0) is fatal (quadrant-3 bug), so
  heads 3/7's dots run as K=64 matmuls at (64,0) against a KT copy
  with the neighbouring head's rows zeroed.
"""

import os
import numpy as np
import ml_dtypes

import concourse.bass as bass
import concourse.tile as tile
from concourse import bacc, mybir
from concourse.bass_utils import run_bass_kernel_spmd
from concourse.tile import add_dep_helper

NPBF16 = ml_dtypes.bfloat16
BF16 = mybir.dt.bfloat16
F32 = mybir.dt.float32

HEADS, DK, DV = 8, 32, 64
N = 1024          # positions = 32*32
C = 256           # channels
IDV = HEADS * DV  # 512
NCORES = 8
BLOC = 2          # batch elems per core
SCALE = float(DK) ** -0.5
EPS = 1e-5

# every 4th post-exp bias multiply runs on GpSimd instead of DVE; its
# consumer (the AV matmul) runs a whole head later, hiding Pool latency.
POOL_EVERY = 1000000

_CACHE = {}


def _build_nc():
    nc = bacc.Bacc("TRN2", target_bir_lowering=False, debug=False)

    xt_d = nc.declare_dram_parameter("xt", [BLOC, 2, 128, N], BF16, isOutput=False)
    wq_d = nc.declare_dram_parameter("wq", [128, 2, C], BF16, isOutput=False)
    wk_d = nc.declare_dram_parameter("wk", [128, 2, C], BF16, isOutput=False)
    wv_d = nc.declare_dram_parameter("wv", [128, 2, IDV], BF16, isOutput=False)
    wo_d = nc.declare_dram_parameter("wo", [128, 4, C], BF16, isOutput=False)
    oq_d = nc.declare_dram_parameter("oq", [128, 2], F32, isOutput=False)
    ok_d = nc.declare_dram_parameter("ok", [128, 2], F32, isOutput=False)
    ovg_d = nc.declare_dram_parameter("ovg", [128, 4], F32, isOutput=False)
    bout_d = nc.declare_dram_parameter("bout", [128, C], F32, isOutput=False)
    # E[h, jt, j1, i] = exp(pos_bias[j, i, h] / SCALE)
    e_d = nc.declare_dram_parameter("etab", [HEADS, 8, 128, N], BF16,
                                    isOutput=False)
    out_d = nc.declare_dram_parameter("out", [BLOC, N, C], F32, isOutput=True)

    Exp = mybir.ActivationFunctionType.Exp
    Gelu = mybir.ActivationFunctionType.Gelu

    with tile.TileContext(nc) as tc:
        with (
            tc.tile_pool(name="const", bufs=1) as const,
            tc.tile_pool(name="persist", bufs=1) as persist,
            tc.tile_pool(name="ep", bufs=3) as ep,        # E tiles in flight
            tc.tile_pool(name="etp", bufs=6) as etp,      # raw exp tiles
            tc.tile_pool(name="et2p", bufs=29) as et2p,   # attn-weight tiles
            tc.tile_pool(name="recp", bufs=3) as recp,
            tc.tile_pool(name="dpsum", bufs=2, space="PSUM") as dpsum,
            tc.tile_pool(name="avp", bufs=4, space="PSUM") as avp,
        ):
            dma = nc.sync

            # critical-path DMAs first: b0's x, then the Q/K weights the
            # first dots need, then everything else.
            xt = {}
            for ct in range(2):
                t = persist.tile([128, N], BF16, tag=f"xt0{ct}",
                                 name=f"xt0{ct}")
                dma.dma_start(t[:], xt_d[0, ct])
                xt[0, ct] = t

            wq_s = const.tile([128, 2, C], BF16, tag="wq")
            dma.dma_start(wq_s[:], wq_d[:])
            oq_s = const.tile([128, 2], F32, tag="oq")
            dma.dma_start(oq_s[:], oq_d[:])
            wk_s = const.tile([128, 2, C], BF16, tag="wk")
            dma.dma_start(wk_s[:], wk_d[:])
            ok_s = const.tile([128, 2], F32, tag="ok")
            dma.dma_start(ok_s[:], ok_d[:])
            for ct in range(2):
                t = persist.tile([128, N], BF16, tag=f"xt1{ct}",
                                 name=f"xt1{ct}")
                dma.dma_start(t[:], xt_d[1, ct])
                xt[1, ct] = t
            wv_s = const.tile([128, 2, IDV], BF16, tag="wv")
            dma.dma_start(wv_s[:], wv_d[:])
            wo_s = const.tile([128, 4, C], BF16, tag="wo")
            dma.dma_start(wo_s[:], wo_d[:])
            ovg_s = const.tile([128, 4], F32, tag="ovg")
            dma.dma_start(ovg_s[:], ovg_d[:])
            bout_s = const.tile([128, C], F32, tag="bout")
            dma.dma_start(bout_s[:], bout_d[:])

            qt, kt, kzt, vt = {}, {}, {}, {}
            for b in range(BLOC):
                for dt in range(2):
                    qt[b, dt] = persist.tile([128, N], BF16, tag=f"qt{b}{dt}",
                                             name=f"qt{b}{dt}")
                    kt[b, dt] = persist.tile([128, N], BF16, tag=f"kt{b}{dt}",
                                             name=f"kt{b}{dt}")
                    kzt[b, dt] = persist.tile([128, N], BF16, tag=f"kz{b}{dt}",
                                              name=f"kz{b}{dt}")

            def qk_unit(b, dt, which):
                """One Q-or-K projection tile: 4 matmuls + DVE assembly.

                Uses the avp psum pool (idle during startup/h0) so the
                dots double-buffer in dpsum is never blocked."""
                wsb, osb = (wq_s, oq_s) if which == "q" else (wk_s, ok_s)
                dst = qt[b, dt] if which == "q" else kt[b, dt]
                for ih in range(2):
                    ps = avp.tile([128, 512], F32, tag="av",
                                  name=f"pps{b}{dt}{which}{ih}")
                    for ct in range(2):
                        nc.tensor.matmul(
                            ps[:],
                            wsb[:, ct, dt * 128:(dt + 1) * 128],
                            xt[b, ct][:, ih * 512:(ih + 1) * 512],
                            start=(ct == 0), stop=(ct == 1),
                        )
                    nc.vector.tensor_scalar_add(
                        dst[:, ih * 512:(ih + 1) * 512], ps[:],
                        osb[:, dt:dt + 1])

            def kz_unit(b, dt):
                """kz = KT with head-2 rows zeroed (cheap all-SBUF copy);
                only needed once heads 3 (dt0) / 7 (dt1) come up."""
                kz = kzt[b, dt]
                nc.gpsimd.memset(kz[64:96, :], 0.0)
                nc.vector.tensor_copy(kz[96:128, :], kt[b, dt][96:128, :])

            def v_unit(b, jt, act_copy=False):
                """One Vaug tile: [j, (h, V|ones 128)]; cols 64:128 = 1.0.

                act_copy routes the psum->sbuf copy through the ACT engine
                (Copy shares the Exp table set, so no table loads); used
                for half the tiles to relieve DVE during startup."""
                v = persist.tile([128, HEADS, 128], BF16, tag=f"v{b}{jt}",
                                 name=f"v{b}{jt}")
                vt[b, jt] = v
                nc.gpsimd.memset(v[:, :, 64:128], 1.0)
                ps = avp.tile([128, 512], F32, tag="av", name=f"vps{b}{jt}")
                for ct in range(2):
                    nc.tensor.matmul(
                        ps[:],
                        xt[b, ct][:, jt * 128:(jt + 1) * 128],
                        wv_s[:, ct, :],
                        start=(ct == 0), stop=(ct == 1),
                    )
                if act_copy:
                    nc.scalar.activation(
                        v[:, :, 0:64],
                        ps[:].rearrange("p (h d) -> p h d", h=HEADS),
                        mybir.ActivationFunctionType.Copy)
                else:
                    nc.vector.tensor_copy(
                        v[:, :, 0:64],
                        ps[:].rearrange("p (h d) -> p h d", h=HEADS))

            # warm the PE p-state during the initial DMA wait: ~3us of
            # matmuls on a zeroed scratch (results never read).
            wscr = persist.tile([128, 512], BF16, tag="wscr", name="wscr")
            nc.gpsimd.memset(wscr[:], 0.0)
            for w in range(6):
                wps = avp.tile([128, 512], F32, tag="av", name=f"warm{w}")
                nc.tensor.matmul(wps[:], wscr[:, 0:128], wscr[:],
                                 start=True, stop=True)

            # E-table quad DMAs, kept 2 in flight ahead of consumption
            equads = [(h, q) for h in range(HEADS) for q in range(2)]
            etabs = {}
            eptr = [0]

            def issue_equad():
                if eptr[0] >= len(equads):
                    return
                h, q = equads[eptr[0]]
                eptr[0] += 1
                t = ep.tile([128, 4, N], BF16, tag="etab", name=f"e{h}{q}")
                etabs[h, q] = t
                dma.dma_start(
                    t[:], e_d[h, 4 * q:4 * q + 4].rearrange("t j i -> j t i"))

            issue_equad()
            issue_equad()

            # b0's dt=0 projections up front (head 0 starts on them); all
            # other projection work rides head 0's loop as extra units.
            qk_unit(0, 0, "q")
            qk_unit(0, 0, "k")
            extras = [lambda: qk_unit(1, 0, "q"), lambda: qk_unit(1, 0, "k")]
            for b in range(BLOC):
                for jt in range(8):
                    extras.append(lambda b=b, jt=jt: v_unit(b, jt, jt % 2 == 0))
            for b in range(BLOC):
                extras.append(lambda b=b: qk_unit(b, 1, "q"))
                extras.append(lambda b=b: qk_unit(b, 1, "k"))
            for b in range(BLOC):
                for dt in range(2):
                    extras.append(lambda b=b, dt=dt: kz_unit(b, dt))
            extras = extras[::-1]

            # gt[b, hp]: gelu-layout attention output [(2 heads x 64 d), i]
            gt = {}
            for b in range(BLOC):
                for hp in range(4):
                    gt[b, hp] = persist.tile([128, N], BF16, tag=f"g{b}{hp}",
                                             name=f"g{b}{hp}")

            last_exp = [None]
            nmul = [0]
            avt = {}

            def emit_av(h, jt, b, et2_prev):
                """AV+sums matmuls for head h, contraction chunk jt."""
                if (h, b, 0) not in avt:
                    for isl in range(2):
                        avt[h, b, isl] = avp.tile([128, 512], F32, tag="av",
                                                  name=f"av{h}{b}{isl}")
                for isl in range(2):
                    nc.tensor.matmul(
                        avt[h, b, isl][:],
                        vt[b, jt][:, h, :],
                        et2_prev[jt, b][:, isl * 512:(isl + 1) * 512],
                        start=(jt == 0), stop=(jt == 7),
                    )

            def emit_norm_piece(h, b, isl):
                """softmax divide: gt rows = av(0:64) * 1/sums(64:128)."""
                tp = avt[h, b, isl]
                rec = recp.tile([64, 512], F32, tag="rec",
                                name=f"rec{h}{b}{isl}")
                nc.vector.reciprocal(rec[:], tp[64:128, :])
                nc.vector.tensor_tensor(
                    gt[b, h // 2][64 * (h % 2):64 * (h % 2) + 64,
                                  isl * 512:(isl + 1) * 512],
                    tp[0:64, :], rec[:],
                    mybir.AluOpType.mult)

            et2_prev = None
            for h in range(HEADS):
                dt, hq = h // 4, h % 4
                et2 = {}
                # AV for head h-1 is drained over this head's early slots
                # (the PE stream has ~0.4us slack per slot there), the
                # normalize pieces over the late slots (spreads DVE load).
                av_q = []
                norm_q = []
                if et2_prev is not None:
                    av_q = [(j2, b2) for j2 in range(8)
                            for b2 in range(BLOC)][::-1]
                    norm_q = [(b2, isl) for b2 in range(BLOC)
                              for isl in range(2)][::-1]
                etq = None
                for jt in range(8):
                    if jt % 4 == 0:
                        etq = etabs.pop((h, jt // 4))
                        issue_equad()
                    for b in range(BLOC):
                        dps = dpsum.tile([128, 2, 512], F32, tag="dps",
                                         name=f"dps{h}{jt}{b}")
                        for ih in range(2):
                            if hq < 3:
                                nc.tensor.matmul(
                                    dps[:, ih, :],
                                    kt[b, dt][32 * hq:32 * hq + 32,
                                              jt * 128:(jt + 1) * 128],
                                    qt[b, dt][32 * hq:32 * hq + 32,
                                              ih * 512:(ih + 1) * 512],
                                    start=True, stop=True,
                                    tile_position=(32 * hq, 0),
                                )
                            else:
                                nc.tensor.matmul(
                                    dps[:, ih, :],
                                    kzt[b, dt][64:128,
                                               jt * 128:(jt + 1) * 128],
                                    qt[b, dt][64:128,
                                              ih * 512:(ih + 1) * 512],
                                    start=True, stop=True,
                                    tile_position=(64, 0),
                                )
                        et = etp.tile([128, 2, 512], BF16, tag="et",
                                      name=f"et{h}{jt}{b}")
                        ae = nc.scalar.activation(et[:], dps[:], Exp, scale=SCALE)
                        last_exp[0] = ae
                        t2 = et2p.tile([128, N], BF16, tag="et2",
                                       name=f"et2_{h}{jt}{b}")
                        et2[jt, b] = t2
                        eng = (nc.gpsimd if nmul[0] % POOL_EVERY == POOL_EVERY - 1
                               else nc.vector)
                        nmul[0] += 1
                        eng.tensor_tensor(
                            t2[:].rearrange("p (a f) -> p a f", a=2),
                            et[:],
                            etq[:, jt % 4, :].rearrange("p (a f) -> p a f",
                                                        a=2),
                            mybir.AluOpType.mult)
                        slot = 2 * jt + b
                        if av_q:
                            for _ in range(2):
                                if av_q:
                                    j2, b2 = av_q.pop()
                                    emit_av(h - 1, j2, b2, et2_prev)
                        elif extras:
                            # all extra units must finish inside h0: their
                            # psum slots come from avp, which AV(h0) claims
                            # at the start of h1.
                            extras.pop()()
                            if (slot == 0 or slot >= 4) and extras:
                                extras.pop()()
                        if not av_q and norm_q and slot >= 10:
                            b2, isl = norm_q.pop()
                            emit_norm_piece(h - 1, b2, isl)
                while av_q:
                    j2, b2 = av_q.pop()
                    emit_av(h - 1, j2, b2, et2_prev)
                while norm_q:
                    b2, isl = norm_q.pop()
                    emit_norm_piece(h - 1, b2, isl)
                while et2_prev is None and extras:
                    extras.pop()()
                et2_prev = et2

            # tail: last head's AV + norm, each (b, isl) chain normalized
            # as soon as its 8 accumulation matmuls finish
            h7 = HEADS - 1
            for b in range(BLOC):
                for isl in range(2):
                    avt[h7, b, isl] = avp.tile([128, 512], F32, tag="av",
                                               name=f"av{h7}{b}{isl}")
            for jt in range(8):
                for b in range(BLOC):
                    for isl in range(2):
                        nc.tensor.matmul(
                            avt[h7, b, isl][:],
                            vt[b, jt][:, h7, :],
                            et2_prev[jt, b][:, isl * 512:(isl + 1) * 512],
                            start=(jt == 0), stop=(jt == 7),
                        )
            for b in range(BLOC):
                for isl in range(2):
                    emit_norm_piece(h7, b, isl)

            # ---------------- GELU (+BN_v offset) + out projection ----------
            for b in range(BLOC):
                for hp in range(4):
                    gi = nc.scalar.activation(gt[b, hp][:], gt[b, hp][:], Gelu,
                                              bias=ovg_s[:, hp:hp + 1],
                                              scale=1.0)
                    if last_exp[0] is not None:
                        add_dep_helper(gi.ins, last_exp[0].ins, sync=False,
                                       reason="group ACT table sets")
                osb = persist.tile([128, 8, C], F32, tag=f"osb{b}",
                                   name=f"osb{b}")
                for it in range(8):
                    ops = avp.tile([128, 512], F32, tag="av", name=f"op{b}{it}")
                    for hp in range(4):
                        nc.tensor.matmul(
                            ops[:, 0:C],
                            gt[b, hp][:, it * 128:(it + 1) * 128],
                            wo_s[:, hp, :],
                            start=(hp == 0), stop=(hp == 3),
                        )
                    nc.vector.tensor_tensor(osb[:, it, :], ops[:, 0:C],
                                            bout_s[:], mybir.AluOpType.add)
                    if it % 2 == 1:
                        dma.dma_start(
                            out_d[b, 128 * (it - 1):128 * (it + 1)].rearrange(
                                "(t i) c -> i t c", t=2),
                            osb[:, it - 1:it + 1, :])

    nc.compile()
    return nc


def _host_prep(x, w_q, bn_q, w_k, bn_k, w_v, bn_v, w_out, b_out, bn_out,
               pos_table):
    """Fold BN into weights, build exp-bias table, shard across cores."""
    def fold(bn):
        g, b_, m, v = [np.asarray(a, np.float64) for a in bn]
        s = g / np.sqrt(v + EPS)
        return s, b_ - m * s

    sq, oq = fold(bn_q)
    sk, ok = fold(bn_k)
    sv, ov = fold(bn_v)
    so, oo = fold(bn_out)

    def wtile(w, s, ncols):
        w_eff = (np.asarray(w, np.float64) * s[None, :]).astype(np.float32)
        return np.ascontiguousarray(
            w_eff.reshape(-1, 128, ncols).transpose(1, 0, 2)).astype(NPBF16)

    wq = wtile(w_q, sq, C)
    wk = wtile(w_k, sk, C)
    wv = wtile(w_v, sv, IDV)
    wo = wtile(w_out, so, C)

    oq_t = np.ascontiguousarray(oq.astype(np.float32).reshape(2, 128).T)
    ok_t = np.ascontiguousarray(ok.astype(np.float32).reshape(2, 128).T)
    ovg_t = np.ascontiguousarray(ov.astype(np.float32).reshape(4, 128).T)
    bout_eff = (np.asarray(b_out, np.float64) * so + oo).astype(np.float32)
    bout_t = np.ascontiguousarray(np.broadcast_to(bout_eff, (128, C)))

    # E[h, jt, j1, i] = exp(bias[j, i, h] / SCALE)
    r = np.arange(32)
    pos = np.stack(np.meshgrid(r, r, indexing="ij"), axis=-1).reshape(-1, 2)
    rel = np.abs(pos[:, None, :] - pos[None, :, :])
    idx = rel[..., 0] * 32 + rel[..., 1]                 # [j, i]
    bias = np.asarray(pos_table, np.float32)[idx]        # [j, i, 8]
    etab = np.exp(bias / SCALE).transpose(2, 0, 1)       # [8, j, i]
    etab = np.ascontiguousarray(
        etab.reshape(HEADS, 8, 128, N)).astype(NPBF16)

    x = np.asarray(x, np.float32).reshape(-1, N, C)      # [B, n, C]
    common = dict(wq=wq, wk=wk, wv=wv, wo=wo, oq=oq_t, ok=ok_t, ovg=ovg_t,
                  bout=bout_t, etab=etab)
    in_maps = []
    for c in range(NCORES):
        xl = x[c * BLOC:(c + 1) * BLOC]                  # [2, n, C]
        xtl = xl.transpose(0, 2, 1).reshape(BLOC, 2, 128, N).astype(NPBF16)
        in_maps.append(dict(common, xt=np.ascontiguousarray(xtl)))
    return in_maps


def kernel(**inputs):
    if "nc" not in _CACHE:
        _CACHE["nc"] = _build_nc()
    nc = _CACHE["nc"]
    in_maps = _host_prep(**inputs)
    res = run_bass_kernel_spmd(nc, in_maps, core_ids=list(range(NCORES)),
                               trace=bool(int(os.environ.get("KTRACE", "0"))))
    _CACHE["last_result"] = res
    outs = [res.results[c]["out"].reshape(BLOC, 32, 32, C)
            for c in range(NCORES)]
    return np.concatenate(outs, axis=0).astype(np.float32)


if __name__ == "__main__":
    nc = _build_nc()
    print("build + compile OK")


# revision 40
# speedup vs baseline: 1.6920x; 1.0025x over previous
"""Trainium2 Bass kernel: 8-head attention block (BN-folded projections,
relative-position bias, softmax, GELU + output projection).

Sharding: data-parallel over batch across 8 NeuronCores (2 batch elems/core).
All weights / bias tables replicated; no collectives.

Engine-balanced design for the TimelineSim cost model (matmuls charge
output free-size on PE.ENGINE plus ~149ns fixed on PE.SEQ per
matmul+ldweights pair; ldweights engine time is free; no credit for
tile_position packing; engines execute their instruction streams
in order):

  ACT is the irreducible bottleneck: exp of the 2*8*1024*1024 attention
  logits = 131072 free-elems (~109us) + per-instr init + gelu ~= 144us
  of ACT busy. Everything else is kept under that and pipelined so ACT
  never starves:
  - relative-position bias applied as a POST-EXP multiply by the
    precomputed table E = exp(bias/scale) on DVE (bf16 2x mode),
    instead of identity-matmul accumulation on PE. (GpSimd offload was
    tried and reverted: any slow op in the exp->Emult->AV chain poisons
    the in-order PE/ACT pipelines.)
  - softmax denominators come free from AV: stationary is V augmented
    with 64 ones columns ([j, 128] = V_h | ones), so one matmul stream
    yields AV on partitions 0:64 and the softmax sums on 64:128.
    AV+sums: 256 matmuls / 131072 PE cycles.
  - normalize uses partition-shifted DVE ops (recip rows 64:128 ->
    rows 0:64, multiply into the packed gelu-layout tile).
  - software pipelining: head h's AV matmuls are emitted inside head
    h+1's dots/exp loop (PE stream has slack there); the V projection
    and the second Q/K projection tile ride the h0/h1 loops the same
    way; x is DMA'd before the weights.
  - GELUs are dep-forced after all exps so only 2 activation-table
    loads are emitted.

  HW workaround: tile_position (96,0) is fatal (quadrant-3 bug), so
  heads 3/7's dots run as K=64 matmuls at (64,0) against a KT copy
  with the neighbouring head's rows zeroed.
"""

import os
import numpy as np
import ml_dtypes

import concourse.bass as bass
import concourse.tile as tile
from concourse import bacc, mybir
from concourse.bass_utils import run_bass_kernel_spmd
from concourse.tile import add_dep_helper

NPBF16 = ml_dtypes.bfloat16
BF16 = mybir.dt.bfloat16
F32 = mybir.dt.float32

HEADS, DK, DV = 8, 32, 64
N = 1024          # positions = 32*32
C = 256           # channels
IDV = HEADS * DV  # 512
NCORES = 8
BLOC = 2          # batch elems per core
SCALE = float(DK) ** -0.5
EPS = 1e-5

_CACHE = {}


def _build_nc():
    nc = bacc.Bacc("TRN2", target_bir_lowering=False, debug=False)

    xt_d = nc.declare_dram_parameter("xt", [BLOC, 2, 128, N], BF16, isOutput=False)
    wqk_d = nc.declare_dram_parameter("wqk", [128, 2, 2, C], BF16, isOutput=False)
    wv_d = nc.declare_dram_parameter("wv", [128, 2, IDV], BF16, isOutput=False)
    wo_d = nc.declare_dram_parameter("wo", [128, 4, C], BF16, isOutput=False)
    oqk_d = nc.declare_dram_parameter("oqk", [128, 2, 2], F32, isOutput=False)
    ovg_d = nc.declare_dram_parameter("ovg", [128, 4], F32, isOutput=False)
    bout_d = nc.declare_dram_parameter("bout", [128, C], F32, isOutput=False)
    # E[h, jt, j1, i] = exp(pos_bias[j, i, h] / SCALE)
    e_d = nc.declare_dram_parameter("etab", [HEADS, 8, 128, N], BF16,
                                    isOutput=False)
    out_d = nc.declare_dram_parameter("out", [BLOC, N, C], F32, isOutput=True)

    Exp = mybir.ActivationFunctionType.Exp
    Gelu = mybir.ActivationFunctionType.Gelu

    with tile.TileContext(nc) as tc:
        with (
            tc.tile_pool(name="const", bufs=1) as const,
            tc.tile_pool(name="persist", bufs=1) as persist,
            tc.tile_pool(name="ep", bufs=4) as ep,        # E tiles in flight
            tc.tile_pool(name="etp", bufs=6) as etp,      # raw exp tiles
            tc.tile_pool(name="et2p", bufs=24) as et2p,   # attn-weight tiles
            tc.tile_pool(name="recp", bufs=3) as recp,
            tc.tile_pool(name="dpsum", bufs=2, space="PSUM") as dpsum,
            tc.tile_pool(name="avp", bufs=4, space="PSUM") as avp,
        ):
            dma = nc.sync

            # critical-path DMAs first: b0's x, then the Q/K weights the
            # first dots need, then everything else.
            xt = {}
            for ct in range(2):
                t = persist.tile([128, N], BF16, tag=f"xt0{ct}",
                                 name=f"xt0{ct}")
                dma.dma_start(t[:], xt_d[0, ct])
                xt[0, ct] = t

            wqk_s = const.tile([128, 2, 2, C], BF16, tag="wqk")
            dma.dma_start(wqk_s[:], wqk_d[:])
            oqk_s = const.tile([128, 2, 2], F32, tag="oqk")
            dma.dma_start(oqk_s[:], oqk_d[:])
            for ct in range(2):
                t = persist.tile([128, N], BF16, tag=f"xt1{ct}",
                                 name=f"xt1{ct}")
                dma.dma_start(t[:], xt_d[1, ct])
                xt[1, ct] = t
            wv_s = const.tile([128, 2, IDV], BF16, tag="wv")
            dma.dma_start(wv_s[:], wv_d[:])
            wo_s = const.tile([128, 4, C], BF16, tag="wo")
            dma.dma_start(wo_s[:], wo_d[:])
            ovg_s = const.tile([128, 4], F32, tag="ovg")
            dma.dma_start(ovg_s[:], ovg_d[:])
            bout_s = const.tile([128, C], F32, tag="bout")
            dma.dma_start(bout_s[:], bout_d[:])


            qt, kt, kzt, vt = {}, {}, {}, {}
            for b in range(BLOC):
                for dt in range(2):
                    qt[b, dt] = persist.tile([128, N], BF16, tag=f"qt{b}{dt}",
                                             name=f"qt{b}{dt}")
                    kt[b, dt] = persist.tile([128, N], BF16, tag=f"kt{b}{dt}",
                                             name=f"kt{b}{dt}")
                    kzt[b, dt] = persist.tile([128, N], BF16, tag=f"kz{b}{dt}",
                                              name=f"kz{b}{dt}")

            def qk_unit(b, dt, which):
                """One Q-or-K projection tile: 4 matmuls + DVE assembly.

                Uses the avp psum pool (idle during startup/h0) so the
                dots double-buffer in dpsum is never blocked."""
                for ih in range(2):
                    qk_half(b, dt, which, ih)

            def qk_half(b, dt, which, ih):
                w = 0 if which == "q" else 1
                dst = qt[b, dt] if which == "q" else kt[b, dt]
                ps = avp.tile([128, 512], F32, tag="av",
                              name=f"pps{b}{dt}{which}{ih}")
                for ct in range(2):
                    nc.tensor.matmul(
                        ps[:],
                        wqk_s[:, w, ct, dt * 128:(dt + 1) * 128],
                        xt[b, ct][:, ih * 512:(ih + 1) * 512],
                        start=(ct == 0), stop=(ct == 1),
                    )
                nc.vector.tensor_scalar_add(
                    dst[:, ih * 512:(ih + 1) * 512], ps[:],
                    oqk_s[:, w, dt:dt + 1])

            def kz_unit(b, dt):
                """kz = KT with head-2 rows zeroed (cheap all-SBUF copy);
                only needed once heads 3 (dt0) / 7 (dt1) come up."""
                kz = kzt[b, dt]
                nc.gpsimd.memset(kz[64:96, :], 0.0)
                nc.vector.tensor_copy(kz[96:128, :], kt[b, dt][96:128, :])

            def v_unit(b, jt, act_copy=False):
                """One Vaug tile: [j, (h, V|ones 128)]; cols 64:128 = 1.0.

                act_copy routes the psum->sbuf copy through the ACT engine
                (Copy shares the Exp table set, so no table loads); used
                for half the tiles to relieve DVE during startup."""
                v = persist.tile([128, HEADS, 128], BF16, tag=f"v{b}{jt}",
                                 name=f"v{b}{jt}")
                vt[b, jt] = v
                nc.gpsimd.memset(v[:, :, 64:128], 1.0)
                ps = avp.tile([128, 512], F32, tag="av", name=f"vps{b}{jt}")
                for ct in range(2):
                    nc.tensor.matmul(
                        ps[:],
                        xt[b, ct][:, jt * 128:(jt + 1) * 128],
                        wv_s[:, ct, :],
                        start=(ct == 0), stop=(ct == 1),
                    )
                if act_copy:
                    nc.scalar.activation(
                        v[:, :, 0:64],
                        ps[:].rearrange("p (h d) -> p h d", h=HEADS),
                        mybir.ActivationFunctionType.Copy)
                else:
                    nc.vector.tensor_copy(
                        v[:, :, 0:64],
                        ps[:].rearrange("p (h d) -> p h d", h=HEADS))

            # warm the PE p-state during the initial DMA wait: ~3us of
            # matmuls on a zeroed scratch (results never read).
            wscr = persist.tile([128, 512], BF16, tag="wscr", name="wscr")
            nc.gpsimd.memset(wscr[:], 0.0)
            for w in range(6):
                wps = avp.tile([128, 512], F32, tag="av", name=f"warm{w}")
                nc.tensor.matmul(wps[:], wscr[:, 0:128], wscr[:],
                                 start=True, stop=True)

            # E-table quad DMAs, kept 2 in flight ahead of consumption
            equads = [(h, q) for h in range(HEADS) for q in range(2)]
            etabs = {}
            eptr = [0]

            def issue_equad():
                if eptr[0] >= len(equads):
                    return
                h, q = equads[eptr[0]]
                eptr[0] += 1
                t = ep.tile([128, 4, N], BF16, tag="etab", name=f"e{h}{q}")
                etabs[h, q] = t
                dma.dma_start(
                    t[:], e_d[h, 4 * q:4 * q + 4].rearrange("t j i -> j t i"))

            issue_equad()
            issue_equad()

            # b0's dt=0 projections up front (head 0 starts on them); all
            # other projection work rides head 0's loop as extra units.
            qk_half(0, 0, "q", 0)
            qk_half(0, 0, "k", 0)
            qk_half(0, 0, "q", 1)
            qk_half(0, 0, "k", 1)
            extras = [lambda: qk_unit(1, 0, "q"), lambda: qk_unit(1, 0, "k")]
            for b in range(BLOC):
                for jt in range(8):
                    extras.append(lambda b=b, jt=jt: v_unit(b, jt, jt % 2 == 0))
            for b in range(BLOC):
                extras.append(lambda b=b: qk_unit(b, 1, "q"))
                extras.append(lambda b=b: qk_unit(b, 1, "k"))
            for b in range(BLOC):
                for dt in range(2):
                    extras.append(lambda b=b, dt=dt: kz_unit(b, dt))
            extras = extras[::-1]

            # gt[b, hp]: gelu-layout attention output [(2 heads x 64 d), i]
            gt = {}
            for b in range(BLOC):
                for hp in range(4):
                    gt[b, hp] = persist.tile([128, N], BF16, tag=f"g{b}{hp}",
                                             name=f"g{b}{hp}")

            last_exp = [None]
            avt = {}

            def emit_av(h, jt, b, et2_prev):
                """AV+sums matmuls for head h, contraction chunk jt."""
                if (h, b, 0) not in avt:
                    for isl in range(2):
                        avt[h, b, isl] = avp.tile([128, 512], F32, tag="av",
                                                  name=f"av{h}{b}{isl}")
                for isl in range(2):
                    nc.tensor.matmul(
                        avt[h, b, isl][:],
                        vt[b, jt][:, h, :],
                        et2_prev[jt, b][:, isl * 512:(isl + 1) * 512],
                        start=(jt == 0), stop=(jt == 7),
                    )

            def emit_norm_piece(h, b, isl):
                """softmax divide: gt rows = av(0:64) * 1/sums(64:128)."""
                tp = avt[h, b, isl]
                rec = recp.tile([64, 512], F32, tag="rec",
                                name=f"rec{h}{b}{isl}")
                nc.vector.reciprocal(rec[:], tp[64:128, :])
                nc.vector.tensor_tensor(
                    gt[b, h // 2][64 * (h % 2):64 * (h % 2) + 64,
                                  isl * 512:(isl + 1) * 512],
                    tp[0:64, :], rec[:],
                    mybir.AluOpType.mult)

            et2_prev = None
            for h in range(HEADS):
                dt, hq = h // 4, h % 4
                et2 = {}
                # AV for head h-1 is drained over this head's early slots
                # (the PE stream has ~0.4us slack per slot there), the
                # normalize pieces over the late slots (spreads DVE load).
                av_q = []
                norm_q = []
                if et2_prev is not None:
                    av_q = [(j2, b2) for j2 in range(8)
                            for b2 in range(BLOC)][::-1]
                    norm_q = [(b2, isl) for b2 in range(BLOC)
                              for isl in range(2)][::-1]
                etq = None
                for jt in range(8):
                    if jt % 4 == 0:
                        etq = etabs.pop((h, jt // 4))
                        issue_equad()
                    for b in range(BLOC):
                        dps = dpsum.tile([128, 2, 512], F32, tag="dps",
                                         name=f"dps{h}{jt}{b}")
                        for ih in range(2):
                            if hq < 3:
                                nc.tensor.matmul(
                                    dps[:, ih, :],
                                    kt[b, dt][32 * hq:32 * hq + 32,
                                              jt * 128:(jt + 1) * 128],
                                    qt[b, dt][32 * hq:32 * hq + 32,
                                              ih * 512:(ih + 1) * 512],
                                    start=True, stop=True,
                                    tile_position=(32 * hq, 0),
                                )
                            else:
                                nc.tensor.matmul(
                                    dps[:, ih, :],
                                    kzt[b, dt][64:128,
                                               jt * 128:(jt + 1) * 128],
                                    qt[b, dt][64:128,
                                              ih * 512:(ih + 1) * 512],
                                    start=True, stop=True,
                                    tile_position=(64, 0),
                                )
                        et = etp.tile([128, 2, 512], BF16, tag="et",
                                      name=f"et{h}{jt}{b}")
                        ae = nc.scalar.activation(et[:], dps[:], Exp, scale=SCALE)
                        last_exp[0] = ae
                        t2 = et2p.tile([128, N], BF16, tag="et2",
                                       name=f"et2_{h}{jt}{b}")
                        et2[jt, b] = t2
                        nc.vector.tensor_tensor(
                            t2[:].rearrange("p (a f) -> p a f", a=2),
                            et[:],
                            etq[:, jt % 4, :].rearrange("p (a f) -> p a f",
                                                        a=2),
                            mybir.AluOpType.mult)
                        slot = 2 * jt + b
                        if av_q:
                            for _ in range(2):
                                if av_q:
                                    j2, b2 = av_q.pop()
                                    emit_av(h - 1, j2, b2, et2_prev)
                        elif extras:
                            # all extra units must finish inside h0: their
                            # psum slots come from avp, which AV(h0) claims
                            # at the start of h1.
                            extras.pop()()
                            if (slot == 0 or slot >= 4) and extras:
                                extras.pop()()
                        if not av_q and norm_q and slot >= 10:
                            b2, isl = norm_q.pop()
                            emit_norm_piece(h - 1, b2, isl)
                while av_q:
                    j2, b2 = av_q.pop()
                    emit_av(h - 1, j2, b2, et2_prev)
                while norm_q:
                    b2, isl = norm_q.pop()
                    emit_norm_piece(h - 1, b2, isl)
                while et2_prev is None and extras:
                    extras.pop()()
                et2_prev = et2

            # tail: last head's AV + norm, each (b, isl) chain normalized
            # as soon as its 8 accumulation matmuls finish
            h7 = HEADS - 1
            for b in range(BLOC):
                for isl in range(2):
                    avt[h7, b, isl] = avp.tile([128, 512], F32, tag="av",
                                               name=f"av{h7}{b}{isl}")
            for jt in range(8):
                for b in range(BLOC):
                    for isl in range(2):
                        nc.tensor.matmul(
                            avt[h7, b, isl][:],
                            vt[b, jt][:, h7, :],
                            et2_prev[jt, b][:, isl * 512:(isl + 1) * 512],
                            start=(jt == 0), stop=(jt == 7),
                        )
            for b in range(BLOC):
                for isl in range(2):
                    emit_norm_piece(h7, b, isl)

            # ---------------- GELU (+BN_v offset) + out projection ----------
            for b in range(BLOC):
                for hp in range(4):
                    gi = nc.scalar.activation(gt[b, hp][:], gt[b, hp][:], Gelu,
                                              bias=ovg_s[:, hp:hp + 1],
                                              scale=1.0)
                    if last_exp[0] is not None:
                        add_dep_helper(gi.ins, last_exp[0].ins, sync=False,
                                       reason="group ACT table sets")
                osb = persist.tile([128, 8, C], F32, tag=f"osb{b}",
                                   name=f"osb{b}")
                for it in range(8):
                    ops = avp.tile([128, 512], F32, tag="av", name=f"op{b}{it}")
                    for hp in range(4):
                        nc.tensor.matmul(
                            ops[:, 0:C],
                            gt[b, hp][:, it * 128:(it + 1) * 128],
                            wo_s[:, hp, :],
                            start=(hp == 0), stop=(hp == 3),
                        )
                    nc.vector.tensor_tensor(osb[:, it, :], ops[:, 0:C],
                                            bout_s[:], mybir.AluOpType.add)
                    if it % 2 == 1:
                        dma.dma_start(
                            out_d[b, 128 * (it - 1):128 * (it + 1)].rearrange(
                                "(t i) c -> i t c", t=2),
                            osb[:, it - 1:it + 1, :])

    nc.compile()
    return nc


def _host_prep(x, w_q, bn_q, w_k, bn_k, w_v, bn_v, w_out, b_out, bn_out,
               pos_table):
    """Fold BN into weights, build exp-bias table, shard across cores."""
    def fold(bn):
        g, b_, m, v = [np.asarray(a, np.float64) for a in bn]
        s = g / np.sqrt(v + EPS)
        return s, b_ - m * s

    sq, oq = fold(bn_q)
    sk, ok = fold(bn_k)
    sv, ov = fold(bn_v)
    so, oo = fold(bn_out)

    def wtile(w, s, ncols):
        w_eff = (np.asarray(w, np.float64) * s[None, :]).astype(np.float32)
        return np.ascontiguousarray(
            w_eff.reshape(-1, 128, ncols).transpose(1, 0, 2)).astype(NPBF16)

    wqk = np.ascontiguousarray(
        np.stack([wtile(w_q, sq, C), wtile(w_k, sk, C)], axis=1))
    wv = wtile(w_v, sv, IDV)
    wo = wtile(w_out, so, C)

    oqk_t = np.ascontiguousarray(np.stack(
        [oq.astype(np.float32).reshape(2, 128).T,
         ok.astype(np.float32).reshape(2, 128).T], axis=1))
    ovg_t = np.ascontiguousarray(ov.astype(np.float32).reshape(4, 128).T)
    bout_eff = (np.asarray(b_out, np.float64) * so + oo).astype(np.float32)
    bout_t = np.ascontiguousarray(np.broadcast_to(bout_eff, (128, C)))

    # E[h, jt, j1, i] = exp(bias[j, i, h] / SCALE)
    r = np.arange(32)
    pos = np.stack(np.meshgrid(r, r, indexing="ij"), axis=-1).reshape(-1, 2)
    rel = np.abs(pos[:, None, :] - pos[None, :, :])
    idx = rel[..., 0] * 32 + rel[..., 1]                 # [j, i]
    bias = np.asarray(pos_table, np.float32)[idx]        # [j, i, 8]
    etab = np.exp(bias / SCALE).transpose(2, 0, 1)       # [8, j, i]
    etab = np.ascontiguousarray(
        etab.reshape(HEADS, 8, 128, N)).astype(NPBF16)

    x = np.asarray(x, np.float32).reshape(-1, N, C)      # [B, n, C]
    common = dict(wqk=wqk, wv=wv, wo=wo, oqk=oqk_t, ovg=ovg_t,
                  bout=bout_t, etab=etab)
    in_maps = []
    for c in range(NCORES):
        xl = x[c * BLOC:(c + 1) * BLOC]                  # [2, n, C]
        xtl = xl.transpose(0, 2, 1).reshape(BLOC, 2, 128, N).astype(NPBF16)
        in_maps.append(dict(common, xt=np.ascontiguousarray(xtl)))
    return in_maps


def kernel(**inputs):
    if "nc" not in _CACHE:
        _CACHE["nc"] = _build_nc()
    nc = _CACHE["nc"]
    in_maps = _host_prep(**inputs)
    res = run_bass_kernel_spmd(nc, in_maps, core_ids=list(range(NCORES)),
                               trace=bool(int(os.environ.get("KTRACE", "0"))))
    _CACHE["last_result"] = res
    outs = [res.results[c]["out"].reshape(BLOC, 32, 32, C)
            for c in range(NCORES)]
    return np.concatenate(outs, axis=0).astype(np.float32)


if __name__ == "__main__":
    nc = _build_nc()
    print("build + compile OK")


# revision 49
# speedup vs baseline: 1.6953x; 1.0019x over previous
"""Trainium2 Bass kernel: 8-head attention block (BN-folded projections,
relative-position bias, softmax, GELU + output projection).

Sharding: data-parallel over batch across 8 NeuronCores (2 batch elems/core).
All weights / bias tables replicated; no collectives.

Engine-balanced design for the TimelineSim cost model (matmuls charge
output free-size on PE.ENGINE plus ~149ns fixed on PE.SEQ per
matmul+ldweights pair; ldweights engine time is free; no credit for
tile_position packing; engines execute their instruction streams
in order):

  ACT is the irreducible bottleneck: exp of the 2*8*1024*1024 attention
  logits = 131072 free-elems (~109us) + per-instr init + gelu ~= 144us
  of ACT busy. Everything else is kept under that and pipelined so ACT
  never starves:
  - relative-position bias applied as a POST-EXP multiply by the
    precomputed table E = exp(bias/scale) on DVE (bf16 2x mode),
    instead of identity-matmul accumulation on PE. (GpSimd offload was
    tried and reverted: any slow op in the exp->Emult->AV chain poisons
    the in-order PE/ACT pipelines.)
  - softmax denominators come free from AV: stationary is V augmented
    with 64 ones columns ([j, 128] = V_h | ones), so one matmul stream
    yields AV on partitions 0:64 and the softmax sums on 64:128.
    AV+sums: 256 matmuls / 131072 PE cycles.
  - normalize uses partition-shifted DVE ops (recip rows 64:128 ->
    rows 0:64, multiply into the packed gelu-layout tile).
  - software pipelining: head h's AV matmuls are emitted inside head
    h+1's dots/exp loop (PE stream has slack there); the V projection
    and the second Q/K projection tile ride the h0/h1 loops the same
    way; x is DMA'd before the weights.
  - GELUs are dep-forced after all exps so only 2 activation-table
    loads are emitted.

  HW workaround: tile_position (96,0) is fatal (quadrant-3 bug), so
  heads 3/7's dots run as K=64 matmuls at (64,0) against a KT copy
  with the neighbouring head's rows zeroed.
"""

import os
import numpy as np
import ml_dtypes

import concourse.bass as bass
import concourse.tile as tile
from concourse import bacc, mybir
from concourse.bass_utils import run_bass_kernel_spmd
from concourse.tile import add_dep_helper

NPBF16 = ml_dtypes.bfloat16
BF16 = mybir.dt.bfloat16
F32 = mybir.dt.float32

HEADS, DK, DV = 8, 32, 64
N = 1024          # positions = 32*32
C = 256           # channels
IDV = HEADS * DV  # 512
NCORES = 8
BLOC = 2          # batch elems per core
SCALE = float(DK) ** -0.5
EPS = 1e-5

_CACHE = {}


def _build_nc():
    nc = bacc.Bacc("TRN2", target_bir_lowering=False, debug=False)

    xt_d = nc.declare_dram_parameter("xt", [BLOC, 2, 128, N], BF16, isOutput=False)
    wqk_d = nc.declare_dram_parameter("wqk", [2, 128, 2, 2, 128], BF16,
                                      isOutput=False)
    wv_d = nc.declare_dram_parameter("wv", [128, 2, IDV], BF16, isOutput=False)
    wo_d = nc.declare_dram_parameter("wo", [128, 4, C], BF16, isOutput=False)
    oqk_d = nc.declare_dram_parameter("oqk", [128, 2, 2], F32, isOutput=False)
    ovg_d = nc.declare_dram_parameter("ovg", [128, 4], F32, isOutput=False)
    bout_d = nc.declare_dram_parameter("bout", [128, C], F32, isOutput=False)
    # E[h, jt, j1, i] = exp(pos_bias[j, i, h] / SCALE)
    e_d = nc.declare_dram_parameter("etab", [HEADS, 8, 128, N], BF16,
                                    isOutput=False)
    out_d = nc.declare_dram_parameter("out", [BLOC, N, C], F32, isOutput=True)

    Exp = mybir.ActivationFunctionType.Exp
    Gelu = mybir.ActivationFunctionType.Gelu

    with tile.TileContext(nc) as tc:
        with (
            tc.tile_pool(name="const", bufs=1) as const,
            tc.tile_pool(name="persist", bufs=1) as persist,
            tc.tile_pool(name="ep", bufs=4) as ep,        # E tiles in flight
            tc.tile_pool(name="etp", bufs=6) as etp,      # raw exp tiles
            tc.tile_pool(name="et2p", bufs=24) as et2p,   # attn-weight tiles
            tc.tile_pool(name="recp", bufs=3) as recp,
            tc.tile_pool(name="dpsum", bufs=2, space="PSUM") as dpsum,
            tc.tile_pool(name="avp", bufs=4, space="PSUM") as avp,
        ):
            dma = nc.sync

            # critical-path DMAs first: b0's x, then the Q/K weights the
            # first dots need, then everything else.
            xt = {}
            for b in range(BLOC):
                for ct in range(2):
                    xt[b, ct] = persist.tile([128, N], BF16, tag=f"xt{b}{ct}",
                                             name=f"xt{b}{ct}")
            wqk_s = const.tile([128, 2, 2, 2, 128], BF16, tag="wqk")
            dma.dma_start(xt[0, 0][:], xt_d[0, 0])
            dma.dma_start(xt[0, 1][:], xt_d[0, 1])
            dma.dma_start(wqk_s[:, 0], wqk_d[0])
            oqk_s = const.tile([128, 2, 2], F32, tag="oqk")
            dma.dma_start(oqk_s[:], oqk_d[:])
            dma.dma_start(xt[1, 0][:], xt_d[1, 0])
            dma.dma_start(xt[1, 1][:], xt_d[1, 1])
            dma.dma_start(wqk_s[:, 1], wqk_d[1])
            wv_s = const.tile([128, 2, IDV], BF16, tag="wv")
            dma.dma_start(wv_s[:], wv_d[:])
            wo_s = const.tile([128, 4, C], BF16, tag="wo")
            dma.dma_start(wo_s[:], wo_d[:])
            ovg_s = const.tile([128, 4], F32, tag="ovg")
            dma.dma_start(ovg_s[:], ovg_d[:])
            bout_s = const.tile([128, C], F32, tag="bout")
            dma.dma_start(bout_s[:], bout_d[:])


            qt, kt, kzt, vt = {}, {}, {}, {}
            for b in range(BLOC):
                for dt in range(2):
                    qt[b, dt] = persist.tile([128, N], BF16, tag=f"qt{b}{dt}",
                                             name=f"qt{b}{dt}")
                    kt[b, dt] = persist.tile([128, N], BF16, tag=f"kt{b}{dt}",
                                             name=f"kt{b}{dt}")
                    kzt[b, dt] = persist.tile([128, N], BF16, tag=f"kz{b}{dt}",
                                              name=f"kz{b}{dt}")

            def qk_unit(b, dt, which):
                """One Q-or-K projection tile: 4 matmuls + DVE assembly.

                Uses the avp psum pool (idle during startup/h0) so the
                dots double-buffer in dpsum is never blocked."""
                for ih in range(2):
                    qk_half(b, dt, which, ih)

            def qk_half(b, dt, which, ih):
                w = 0 if which == "q" else 1
                dst = qt[b, dt] if which == "q" else kt[b, dt]
                ps = avp.tile([128, 512], F32, tag="av",
                              name=f"pps{b}{dt}{which}{ih}")
                for ct in range(2):
                    nc.tensor.matmul(
                        ps[:],
                        wqk_s[:, dt, w, ct, :],
                        xt[b, ct][:, ih * 512:(ih + 1) * 512],
                        start=(ct == 0), stop=(ct == 1),
                    )
                nc.vector.tensor_scalar_add(
                    dst[:, ih * 512:(ih + 1) * 512], ps[:],
                    oqk_s[:, w, dt:dt + 1])

            def kz_unit(b, dt):
                """kz = KT with head-2 rows zeroed (cheap all-SBUF copy);
                only needed once heads 3 (dt0) / 7 (dt1) come up."""
                kz = kzt[b, dt]
                nc.gpsimd.memset(kz[64:96, :], 0.0)
                nc.vector.tensor_copy(kz[96:128, :], kt[b, dt][96:128, :])

            def v_unit(b, jt, act_copy=False):
                """One Vaug tile: [j, (h, V|ones 128)]; cols 64:128 = 1.0.

                act_copy routes the psum->sbuf copy through the ACT engine
                (Copy shares the Exp table set, so no table loads); used
                for half the tiles to relieve DVE during startup."""
                v = persist.tile([128, HEADS, 128], BF16, tag=f"v{b}{jt}",
                                 name=f"v{b}{jt}")
                vt[b, jt] = v
                nc.gpsimd.memset(v[:, :, 64:128], 1.0)
                ps = avp.tile([128, 512], F32, tag="av", name=f"vps{b}{jt}")
                for ct in range(2):
                    nc.tensor.matmul(
                        ps[:],
                        xt[b, ct][:, jt * 128:(jt + 1) * 128],
                        wv_s[:, ct, :],
                        start=(ct == 0), stop=(ct == 1),
                    )
                if act_copy:
                    nc.scalar.activation(
                        v[:, :, 0:64],
                        ps[:].rearrange("p (h d) -> p h d", h=HEADS),
                        mybir.ActivationFunctionType.Copy)
                else:
                    nc.vector.tensor_copy(
                        v[:, :, 0:64],
                        ps[:].rearrange("p (h d) -> p h d", h=HEADS))

            # warm the PE p-state during the initial DMA wait: ~3us of
            # matmuls on a zeroed scratch (results never read).
            wscr = persist.tile([128, 512], BF16, tag="wscr", name="wscr")
            nc.gpsimd.memset(wscr[:], 0.0)
            for w in range(6):
                wps = avp.tile([128, 512], F32, tag="av", name=f"warm{w}")
                nc.tensor.matmul(wps[:], wscr[:, 0:128], wscr[:],
                                 start=True, stop=True)

            # E-table quad DMAs, kept 2 in flight ahead of consumption
            equads = [(h, q) for h in range(HEADS) for q in range(2)]
            etabs = {}
            eptr = [0]

            def issue_equad():
                if eptr[0] >= len(equads):
                    return
                h, q = equads[eptr[0]]
                eptr[0] += 1
                t = ep.tile([128, 4, N], BF16, tag="etab", name=f"e{h}{q}")
                etabs[h, q] = t
                dma.dma_start(
                    t[:], e_d[h, 4 * q:4 * q + 4].rearrange("t j i -> j t i"))

            issue_equad()
            issue_equad()

            # b0's dt=0 projections up front (head 0 starts on them); all
            # other projection work rides head 0's loop as extra units.
            qk_half(0, 0, "q", 0)
            qk_half(0, 0, "k", 0)
            qk_half(0, 0, "q", 1)
            qk_half(0, 0, "k", 1)
            extras = [lambda: qk_unit(1, 0, "q"), lambda: qk_unit(1, 0, "k")]
            for b in range(BLOC):
                for jt in range(8):
                    extras.append(lambda b=b, jt=jt: v_unit(b, jt, jt % 2 == 0))
            for b in range(BLOC):
                extras.append(lambda b=b: qk_unit(b, 1, "q"))
                extras.append(lambda b=b: qk_unit(b, 1, "k"))
            for b in range(BLOC):
                for dt in range(2):
                    extras.append(lambda b=b, dt=dt: kz_unit(b, dt))
            extras = extras[::-1]

            # gt[b, hp]: gelu-layout attention output [(2 heads x 64 d), i]
            gt = {}
            for b in range(BLOC):
                for hp in range(4):
                    gt[b, hp] = persist.tile([128, N], BF16, tag=f"g{b}{hp}",
                                             name=f"g{b}{hp}")

            last_exp = [None]
            avt = {}

            def emit_av(h, jt, b, et2_prev):
                """AV+sums matmuls for head h, contraction chunk jt."""
                if (h, b, 0) not in avt:
                    for isl in range(2):
                        avt[h, b, isl] = avp.tile([128, 512], F32, tag="av",
                                                  name=f"av{h}{b}{isl}")
                for isl in range(2):
                    nc.tensor.matmul(
                        avt[h, b, isl][:],
                        vt[b, jt][:, h, :],
                        et2_prev[jt, b][:, isl * 512:(isl + 1) * 512],
                        start=(jt == 0), stop=(jt == 7),
                    )

            def emit_norm_piece(h, b, isl):
                """softmax divide: gt rows = av(0:64) * 1/sums(64:128)."""
                tp = avt[h, b, isl]
                rec = recp.tile([64, 512], F32, tag="rec",
                                name=f"rec{h}{b}{isl}")
                nc.vector.reciprocal(rec[:], tp[64:128, :])
                nc.vector.tensor_tensor(
                    gt[b, h // 2][64 * (h % 2):64 * (h % 2) + 64,
                                  isl * 512:(isl + 1) * 512],
                    tp[0:64, :], rec[:],
                    mybir.AluOpType.mult)

            et2_prev = None
            for h in range(HEADS):
                dt, hq = h // 4, h % 4
                et2 = {}
                # AV for head h-1 is drained over this head's early slots
                # (the PE stream has ~0.4us slack per slot there), the
                # normalize pieces over the late slots (spreads DVE load).
                av_q = []
                norm_q = []
                if et2_prev is not None:
                    av_q = [(j2, b2) for j2 in range(8)
                            for b2 in range(BLOC)][::-1]
                    norm_q = [(b2, isl) for b2 in range(BLOC)
                              for isl in range(2)][::-1]
                etq = None
                for jt in range(8):
                    if jt % 4 == 0:
                        etq = etabs.pop((h, jt // 4))
                        issue_equad()
                    for b in range(BLOC):
                        dps = dpsum.tile([128, 2, 512], F32, tag="dps",
                                         name=f"dps{h}{jt}{b}")
                        for ih in range(2):
                            if hq < 3:
                                nc.tensor.matmul(
                                    dps[:, ih, :],
                                    kt[b, dt][32 * hq:32 * hq + 32,
                                              jt * 128:(jt + 1) * 128],
                                    qt[b, dt][32 * hq:32 * hq + 32,
                                              ih * 512:(ih + 1) * 512],
                                    start=True, stop=True,
                                    tile_position=(32 * hq, 0),
                                )
                            else:
                                nc.tensor.matmul(
                                    dps[:, ih, :],
                                    kzt[b, dt][64:128,
                                               jt * 128:(jt + 1) * 128],
                                    qt[b, dt][64:128,
                                              ih * 512:(ih + 1) * 512],
                                    start=True, stop=True,
                                    tile_position=(64, 0),
                                )
                        et = etp.tile([128, 2, 512], BF16, tag="et",
                                      name=f"et{h}{jt}{b}")
                        ae = nc.scalar.activation(et[:], dps[:], Exp, scale=SCALE)
                        last_exp[0] = ae
                        t2 = et2p.tile([128, N], BF16, tag="et2",
                                       name=f"et2_{h}{jt}{b}")
                        et2[jt, b] = t2
                        nc.vector.tensor_tensor(
                            t2[:].rearrange("p (a f) -> p a f", a=2),
                            et[:],
                            etq[:, jt % 4, :].rearrange("p (a f) -> p a f",
                                                        a=2),
                            mybir.AluOpType.mult)
                        slot = 2 * jt + b
                        if av_q:
                            for _ in range(2):
                                if av_q:
                                    j2, b2 = av_q.pop()
                                    emit_av(h - 1, j2, b2, et2_prev)
                        elif extras:
                            # all extra units must finish inside h0: their
                            # psum slots come from avp, which AV(h0) claims
                            # at the start of h1.
                            extras.pop()()
                            if (slot == 0 or slot >= 4) and extras:
                                extras.pop()()
                        if not av_q and norm_q and slot >= 10:
                            b2, isl = norm_q.pop()
                            emit_norm_piece(h - 1, b2, isl)
                while av_q:
                    j2, b2 = av_q.pop()
                    emit_av(h - 1, j2, b2, et2_prev)
                while norm_q:
                    b2, isl = norm_q.pop()
                    emit_norm_piece(h - 1, b2, isl)
                while et2_prev is None and extras:
                    extras.pop()()
                et2_prev = et2

            # tail: last head's AV + norm, each (b, isl) chain normalized
            # as soon as its 8 accumulation matmuls finish
            h7 = HEADS - 1
            for b in range(BLOC):
                for isl in range(2):
                    avt[h7, b, isl] = avp.tile([128, 512], F32, tag="av",
                                               name=f"av{h7}{b}{isl}")
            for jt in range(8):
                for b in range(BLOC):
                    for isl in range(2):
                        nc.tensor.matmul(
                            avt[h7, b, isl][:],
                            vt[b, jt][:, h7, :],
                            et2_prev[jt, b][:, isl * 512:(isl + 1) * 512],
                            start=(jt == 0), stop=(jt == 7),
                        )
            for b in range(BLOC):
                for isl in range(2):
                    emit_norm_piece(h7, b, isl)

            # ---------------- GELU (+BN_v offset) + out projection ----------
            for b in range(BLOC):
                for hp in range(4):
                    gi = nc.scalar.activation(gt[b, hp][:], gt[b, hp][:], Gelu,
                                              bias=ovg_s[:, hp:hp + 1],
                                              scale=1.0)
                    if last_exp[0] is not None:
                        add_dep_helper(gi.ins, last_exp[0].ins, sync=False,
                                       reason="group ACT table sets")
                osb = persist.tile([128, 8, C], F32, tag=f"osb{b}",
                                   name=f"osb{b}")
                for it in range(8):
                    ops = avp.tile([128, 512], F32, tag="av", name=f"op{b}{it}")
                    for hp in range(4):
                        nc.tensor.matmul(
                            ops[:, 0:C],
                            gt[b, hp][:, it * 128:(it + 1) * 128],
                            wo_s[:, hp, :],
                            start=(hp == 0), stop=(hp == 3),
                        )
                    nc.vector.tensor_tensor(osb[:, it, :], ops[:, 0:C],
                                            bout_s[:], mybir.AluOpType.add)
                    if it % 2 == 1:
                        dma.dma_start(
                            out_d[b, 128 * (it - 1):128 * (it + 1)].rearrange(
                                "(t i) c -> i t c", t=2),
                            osb[:, it - 1:it + 1, :])

    nc.compile()
    return nc


def _host_prep(x, w_q, bn_q, w_k, bn_k, w_v, bn_v, w_out, b_out, bn_out,
               pos_table):
    """Fold BN into weights, build exp-bias table, shard across cores."""
    def fold(bn):
        g, b_, m, v = [np.asarray(a, np.float64) for a in bn]
        s = g / np.sqrt(v + EPS)
        return s, b_ - m * s

    sq, oq = fold(bn_q)
    sk, ok = fold(bn_k)
    sv, ov = fold(bn_v)
    so, oo = fold(bn_out)

    def wtile(w, s, ncols):
        w_eff = (np.asarray(w, np.float64) * s[None, :]).astype(np.float32)
        return np.ascontiguousarray(
            w_eff.reshape(-1, 128, ncols).transpose(1, 0, 2)).astype(NPBF16)

    # [128, ct, C] per q/k -> [dt, 128, qk, ct, 128]
    wqk = np.stack([wtile(w_q, sq, C), wtile(w_k, sk, C)], axis=1)
    wqk = np.ascontiguousarray(
        wqk.reshape(128, 2, 2, 2, 128).transpose(3, 0, 1, 2, 4))
    wv = wtile(w_v, sv, IDV)
    wo = wtile(w_out, so, C)

    oqk_t = np.ascontiguousarray(np.stack(
        [oq.astype(np.float32).reshape(2, 128).T,
         ok.astype(np.float32).reshape(2, 128).T], axis=1))
    ovg_t = np.ascontiguousarray(ov.astype(np.float32).reshape(4, 128).T)
    bout_eff = (np.asarray(b_out, np.float64) * so + oo).astype(np.float32)
    bout_t = np.ascontiguousarray(np.broadcast_to(bout_eff, (128, C)))

    # E[h, jt, j1, i] = exp(bias[j, i, h] / SCALE)
    r = np.arange(32)
    pos = np.stack(np.meshgrid(r, r, indexing="ij"), axis=-1).reshape(-1, 2)
    rel = np.abs(pos[:, None, :] - pos[None, :, :])
    idx = rel[..., 0] * 32 + rel[..., 1]                 # [j, i]
    bias = np.asarray(pos_table, np.float32)[idx]        # [j, i, 8]
    etab = np.exp(bias / SCALE).transpose(2, 0, 1)       # [8, j, i]
    etab = np.ascontiguousarray(
        etab.reshape(HEADS, 8, 128, N)).astype(NPBF16)

    x = np.asarray(x, np.float32).reshape(-1, N, C)      # [B, n, C]
    common = dict(wqk=wqk, wv=wv, wo=wo, oqk=oqk_t, ovg=ovg_t,
                  bout=bout_t, etab=etab)
    in_maps = []
    for c in range(NCORES):
        xl = x[c * BLOC:(c + 1) * BLOC]                  # [2, n, C]
        xtl = xl.transpose(0, 2, 1).reshape(BLOC, 2, 128, N).astype(NPBF16)
        in_maps.append(dict(common, xt=np.ascontiguousarray(xtl)))
    return in_maps


def kernel(**inputs):
    if "nc" not in _CACHE:
        _CACHE["nc"] = _build_nc()
    nc = _CACHE["nc"]
    in_maps = _host_prep(**inputs)
    res = run_bass_kernel_spmd(nc, in_maps, core_ids=list(range(NCORES)),
                               trace=bool(int(os.environ.get("KTRACE", "0"))))
    _CACHE["last_result"] = res
    outs = [res.results[c]["out"].reshape(BLOC, 32, 32, C)
            for c in range(NCORES)]
    return np.concatenate(outs, axis=0).astype(np.float32)


if __name__ == "__main__":
    nc = _build_nc()
    print("build + compile OK")


# revision 50
# speedup vs baseline: 1.6990x; 1.0022x over previous
"""Trainium2 Bass kernel: 8-head attention block (BN-folded projections,
relative-position bias, softmax, GELU + output projection).

Sharding: data-parallel over batch across 8 NeuronCores (2 batch elems/core).
All weights / bias tables replicated; no collectives.

Engine-balanced design for the TimelineSim cost model (matmuls charge
output free-size on PE.ENGINE plus ~149ns fixed on PE.SEQ per
matmul+ldweights pair; ldweights engine time is free; no credit for
tile_position packing; engines execute their instruction streams
in order):

  ACT is the irreducible bottleneck: exp of the 2*8*1024*1024 attention
  logits = 131072 free-elems (~109us) + per-instr init + gelu ~= 144us
  of ACT busy. Everything else is kept under that and pipelined so ACT
  never starves:
  - relative-position bias applied as a POST-EXP multiply by the
    precomputed table E = exp(bias/scale) on DVE (bf16 2x mode),
    instead of identity-matmul accumulation on PE. (GpSimd offload was
    tried and reverted: any slow op in the exp->Emult->AV chain poisons
    the in-order PE/ACT pipelines.)
  - softmax denominators come free from AV: stationary is V augmented
    with 64 ones columns ([j, 128] = V_h | ones), so one matmul stream
    yields AV on partitions 0:64 and the softmax sums on 64:128.
    AV+sums: 256 matmuls / 131072 PE cycles.
  - normalize uses partition-shifted DVE ops (recip rows 64:128 ->
    rows 0:64, multiply into the packed gelu-layout tile).
  - software pipelining: head h's AV matmuls are emitted inside head
    h+1's dots/exp loop (PE stream has slack there); the V projection
    and the second Q/K projection tile ride the h0/h1 loops the same
    way; x is DMA'd before the weights.
  - GELUs are dep-forced after all exps so only 2 activation-table
    loads are emitted.

  HW workaround: tile_position (96,0) is fatal (quadrant-3 bug), so
  heads 3/7's dots run as K=64 matmuls at (64,0) against a KT copy
  with the neighbouring head's rows zeroed.
"""

import os
import numpy as np
import ml_dtypes

import concourse.bass as bass
import concourse.tile as tile
from concourse import bacc, mybir
from concourse.bass_utils import run_bass_kernel_spmd
from concourse.tile import add_dep_helper

NPBF16 = ml_dtypes.bfloat16
BF16 = mybir.dt.bfloat16
F32 = mybir.dt.float32

HEADS, DK, DV = 8, 32, 64
N = 1024          # positions = 32*32
C = 256           # channels
IDV = HEADS * DV  # 512
NCORES = 8
BLOC = 2          # batch elems per core
SCALE = float(DK) ** -0.5
EPS = 1e-5

_CACHE = {}


def _build_nc():
    nc = bacc.Bacc("TRN2", target_bir_lowering=False, debug=False)

    xt_d = nc.declare_dram_parameter("xt", [BLOC, 2, 128, N], BF16, isOutput=False)
    wqk_d = nc.declare_dram_parameter("wqk", [2, 128, 2, 2, 128], BF16,
                                      isOutput=False)
    wv_d = nc.declare_dram_parameter("wv", [128, 2, IDV], BF16, isOutput=False)
    wo_d = nc.declare_dram_parameter("wo", [128, 4, C], BF16, isOutput=False)
    oqk_d = nc.declare_dram_parameter("oqk", [128, 2, 2], F32, isOutput=False)
    ovg_d = nc.declare_dram_parameter("ovg", [128, 4], F32, isOutput=False)
    bout_d = nc.declare_dram_parameter("bout", [128, C], F32, isOutput=False)
    # E[h, jt, j1, i] = exp(pos_bias[j, i, h] / SCALE)
    e_d = nc.declare_dram_parameter("etab", [HEADS, 8, 128, N], BF16,
                                    isOutput=False)
    out_d = nc.declare_dram_parameter("out", [BLOC, N, C], BF16, isOutput=True)

    Exp = mybir.ActivationFunctionType.Exp
    Gelu = mybir.ActivationFunctionType.Gelu

    with tile.TileContext(nc) as tc:
        with (
            tc.tile_pool(name="const", bufs=1) as const,
            tc.tile_pool(name="persist", bufs=1) as persist,
            tc.tile_pool(name="ep", bufs=4) as ep,        # E tiles in flight
            tc.tile_pool(name="etp", bufs=6) as etp,      # raw exp tiles
            tc.tile_pool(name="et2p", bufs=24) as et2p,   # attn-weight tiles
            tc.tile_pool(name="recp", bufs=3) as recp,
            tc.tile_pool(name="dpsum", bufs=2, space="PSUM") as dpsum,
            tc.tile_pool(name="avp", bufs=4, space="PSUM") as avp,
        ):
            dma = nc.sync

            # critical-path DMAs first: b0's x, then the Q/K weights the
            # first dots need, then everything else.
            xt = {}
            for b in range(BLOC):
                for ct in range(2):
                    xt[b, ct] = persist.tile([128, N], BF16, tag=f"xt{b}{ct}",
                                             name=f"xt{b}{ct}")
            wqk_s = const.tile([128, 2, 2, 2, 128], BF16, tag="wqk")
            dma.dma_start(xt[0, 0][:], xt_d[0, 0])
            dma.dma_start(xt[0, 1][:], xt_d[0, 1])
            dma.dma_start(wqk_s[:, 0], wqk_d[0])
            oqk_s = const.tile([128, 2, 2], F32, tag="oqk")
            dma.dma_start(oqk_s[:], oqk_d[:])
            dma.dma_start(xt[1, 0][:], xt_d[1, 0])
            dma.dma_start(xt[1, 1][:], xt_d[1, 1])
            dma.dma_start(wqk_s[:, 1], wqk_d[1])
            wv_s = const.tile([128, 2, IDV], BF16, tag="wv")
            dma.dma_start(wv_s[:], wv_d[:])
            wo_s = const.tile([128, 4, C], BF16, tag="wo")
            dma.dma_start(wo_s[:], wo_d[:])
            ovg_s = const.tile([128, 4], F32, tag="ovg")
            dma.dma_start(ovg_s[:], ovg_d[:])
            bout_s = const.tile([128, C], F32, tag="bout")
            dma.dma_start(bout_s[:], bout_d[:])


            qt, kt, kzt, vt = {}, {}, {}, {}
            for b in range(BLOC):
                for dt in range(2):
                    qt[b, dt] = persist.tile([128, N], BF16, tag=f"qt{b}{dt}",
                                             name=f"qt{b}{dt}")
                    kt[b, dt] = persist.tile([128, N], BF16, tag=f"kt{b}{dt}",
                                             name=f"kt{b}{dt}")
                    kzt[b, dt] = persist.tile([128, N], BF16, tag=f"kz{b}{dt}",
                                              name=f"kz{b}{dt}")

            def qk_unit(b, dt, which):
                """One Q-or-K projection tile: 4 matmuls + DVE assembly.

                Uses the avp psum pool (idle during startup/h0) so the
                dots double-buffer in dpsum is never blocked."""
                for ih in range(2):
                    qk_half(b, dt, which, ih)

            def qk_half(b, dt, which, ih):
                w = 0 if which == "q" else 1
                dst = qt[b, dt] if which == "q" else kt[b, dt]
                ps = avp.tile([128, 512], F32, tag="av",
                              name=f"pps{b}{dt}{which}{ih}")
                for ct in range(2):
                    nc.tensor.matmul(
                        ps[:],
                        wqk_s[:, dt, w, ct, :],
                        xt[b, ct][:, ih * 512:(ih + 1) * 512],
                        start=(ct == 0), stop=(ct == 1),
                    )
                nc.vector.tensor_scalar_add(
                    dst[:, ih * 512:(ih + 1) * 512], ps[:],
                    oqk_s[:, w, dt:dt + 1])

            def kz_unit(b, dt):
                """kz = KT with head-2 rows zeroed (cheap all-SBUF copy);
                only needed once heads 3 (dt0) / 7 (dt1) come up."""
                kz = kzt[b, dt]
                nc.gpsimd.memset(kz[64:96, :], 0.0)
                nc.vector.tensor_copy(kz[96:128, :], kt[b, dt][96:128, :])

            def v_unit(b, jt, act_copy=False):
                """One Vaug tile: [j, (h, V|ones 128)]; cols 64:128 = 1.0.

                act_copy routes the psum->sbuf copy through the ACT engine
                (Copy shares the Exp table set, so no table loads); used
                for half the tiles to relieve DVE during startup."""
                v = persist.tile([128, HEADS, 128], BF16, tag=f"v{b}{jt}",
                                 name=f"v{b}{jt}")
                vt[b, jt] = v
                nc.gpsimd.memset(v[:, :, 64:128], 1.0)
                ps = avp.tile([128, 512], F32, tag="av", name=f"vps{b}{jt}")
                for ct in range(2):
                    nc.tensor.matmul(
                        ps[:],
                        xt[b, ct][:, jt * 128:(jt + 1) * 128],
                        wv_s[:, ct, :],
                        start=(ct == 0), stop=(ct == 1),
                    )
                if act_copy:
                    nc.scalar.activation(
                        v[:, :, 0:64],
                        ps[:].rearrange("p (h d) -> p h d", h=HEADS),
                        mybir.ActivationFunctionType.Copy)
                else:
                    nc.vector.tensor_copy(
                        v[:, :, 0:64],
                        ps[:].rearrange("p (h d) -> p h d", h=HEADS))

            # warm the PE p-state during the initial DMA wait: ~3us of
            # matmuls on a zeroed scratch (results never read).
            wscr = persist.tile([128, 512], BF16, tag="wscr", name="wscr")
            nc.gpsimd.memset(wscr[:], 0.0)
            for w in range(6):
                wps = avp.tile([128, 512], F32, tag="av", name=f"warm{w}")
                nc.tensor.matmul(wps[:], wscr[:, 0:128], wscr[:],
                                 start=True, stop=True)

            # E-table quad DMAs, kept 2 in flight ahead of consumption
            equads = [(h, q) for h in range(HEADS) for q in range(2)]
            etabs = {}
            eptr = [0]

            def issue_equad():
                if eptr[0] >= len(equads):
                    return
                h, q = equads[eptr[0]]
                eptr[0] += 1
                t = ep.tile([128, 4, N], BF16, tag="etab", name=f"e{h}{q}")
                etabs[h, q] = t
                dma.dma_start(
                    t[:], e_d[h, 4 * q:4 * q + 4].rearrange("t j i -> j t i"))

            issue_equad()
            issue_equad()

            # b0's dt=0 projections up front (head 0 starts on them); all
            # other projection work rides head 0's loop as extra units.
            qk_half(0, 0, "q", 0)
            qk_half(0, 0, "k", 0)
            qk_half(0, 0, "q", 1)
            qk_half(0, 0, "k", 1)
            extras = [lambda: qk_unit(1, 0, "q"), lambda: qk_unit(1, 0, "k")]
            for b in range(BLOC):
                for jt in range(8):
                    extras.append(lambda b=b, jt=jt: v_unit(b, jt, jt % 2 == 0))
            for b in range(BLOC):
                extras.append(lambda b=b: qk_unit(b, 1, "q"))
                extras.append(lambda b=b: qk_unit(b, 1, "k"))
            for b in range(BLOC):
                for dt in range(2):
                    extras.append(lambda b=b, dt=dt: kz_unit(b, dt))
            extras = extras[::-1]

            # gt[b, hp]: gelu-layout attention output [(2 heads x 64 d), i]
            gt = {}
            for b in range(BLOC):
                for hp in range(4):
                    gt[b, hp] = persist.tile([128, N], BF16, tag=f"g{b}{hp}",
                                             name=f"g{b}{hp}")

            last_exp = [None]
            avt = {}

            def emit_av(h, jt, b, et2_prev):
                """AV+sums matmuls for head h, contraction chunk jt."""
                if (h, b, 0) not in avt:
                    for isl in range(2):
                        avt[h, b, isl] = avp.tile([128, 512], F32, tag="av",
                                                  name=f"av{h}{b}{isl}")
                for isl in range(2):
                    nc.tensor.matmul(
                        avt[h, b, isl][:],
                        vt[b, jt][:, h, :],
                        et2_prev[jt, b][:, isl * 512:(isl + 1) * 512],
                        start=(jt == 0), stop=(jt == 7),
                    )

            def emit_norm_piece(h, b, isl):
                """softmax divide: gt rows = av(0:64) * 1/sums(64:128)."""
                tp = avt[h, b, isl]
                rec = recp.tile([64, 512], F32, tag="rec",
                                name=f"rec{h}{b}{isl}")
                nc.vector.reciprocal(rec[:], tp[64:128, :])
                nc.vector.tensor_tensor(
                    gt[b, h // 2][64 * (h % 2):64 * (h % 2) + 64,
                                  isl * 512:(isl + 1) * 512],
                    tp[0:64, :], rec[:],
                    mybir.AluOpType.mult)

            et2_prev = None
            for h in range(HEADS):
                dt, hq = h // 4, h % 4
                et2 = {}
                # AV for head h-1 is drained over this head's early slots
                # (the PE stream has ~0.4us slack per slot there), the
                # normalize pieces over the late slots (spreads DVE load).
                av_q = []
                norm_q = []
                if et2_prev is not None:
                    av_q = [(j2, b2) for j2 in range(8)
                            for b2 in range(BLOC)][::-1]
                    norm_q = [(b2, isl) for b2 in range(BLOC)
                              for isl in range(2)][::-1]
                etq = None
                for jt in range(8):
                    if jt % 4 == 0:
                        etq = etabs.pop((h, jt // 4))
                        issue_equad()
                    for b in range(BLOC):
                        dps = dpsum.tile([128, 2, 512], F32, tag="dps",
                                         name=f"dps{h}{jt}{b}")
                        for ih in range(2):
                            if hq < 3:
                                nc.tensor.matmul(
                                    dps[:, ih, :],
                                    kt[b, dt][32 * hq:32 * hq + 32,
                                              jt * 128:(jt + 1) * 128],
                                    qt[b, dt][32 * hq:32 * hq + 32,
                                              ih * 512:(ih + 1) * 512],
                                    start=True, stop=True,
                                    tile_position=(32 * hq, 0),
                                )
                            else:
                                nc.tensor.matmul(
                                    dps[:, ih, :],
                                    kzt[b, dt][64:128,
                                               jt * 128:(jt + 1) * 128],
                                    qt[b, dt][64:128,
                                              ih * 512:(ih + 1) * 512],
                                    start=True, stop=True,
                                    tile_position=(64, 0),
                                )
                        et = etp.tile([128, 2, 512], BF16, tag="et",
                                      name=f"et{h}{jt}{b}")
                        ae = nc.scalar.activation(et[:], dps[:], Exp, scale=SCALE)
                        last_exp[0] = ae
                        t2 = et2p.tile([128, N], BF16, tag="et2",
                                       name=f"et2_{h}{jt}{b}")
                        et2[jt, b] = t2
                        nc.vector.tensor_tensor(
                            t2[:].rearrange("p (a f) -> p a f", a=2),
                            et[:],
                            etq[:, jt % 4, :].rearrange("p (a f) -> p a f",
                                                        a=2),
                            mybir.AluOpType.mult)
                        slot = 2 * jt + b
                        if av_q:
                            for _ in range(2):
                                if av_q:
                                    j2, b2 = av_q.pop()
                                    emit_av(h - 1, j2, b2, et2_prev)
                        elif extras:
                            # all extra units must finish inside h0: their
                            # psum slots come from avp, which AV(h0) claims
                            # at the start of h1.
                            extras.pop()()
                            if (slot == 0 or slot >= 4) and extras:
                                extras.pop()()
                        if not av_q and norm_q and slot >= 10:
                            b2, isl = norm_q.pop()
                            emit_norm_piece(h - 1, b2, isl)
                while av_q:
                    j2, b2 = av_q.pop()
                    emit_av(h - 1, j2, b2, et2_prev)
                while norm_q:
                    b2, isl = norm_q.pop()
                    emit_norm_piece(h - 1, b2, isl)
                while et2_prev is None and extras:
                    extras.pop()()
                et2_prev = et2

            # tail: last head's AV + norm, each (b, isl) chain normalized
            # as soon as its 8 accumulation matmuls finish
            h7 = HEADS - 1
            for b in range(BLOC):
                for isl in range(2):
                    avt[h7, b, isl] = avp.tile([128, 512], F32, tag="av",
                                               name=f"av{h7}{b}{isl}")
            for jt in range(8):
                for b in range(BLOC):
                    for isl in range(2):
                        nc.tensor.matmul(
                            avt[h7, b, isl][:],
                            vt[b, jt][:, h7, :],
                            et2_prev[jt, b][:, isl * 512:(isl + 1) * 512],
                            start=(jt == 0), stop=(jt == 7),
                        )
            for b in range(BLOC):
                for isl in range(2):
                    emit_norm_piece(h7, b, isl)

            # ---------------- GELU (+BN_v offset) + out projection ----------
            for b in range(BLOC):
                for hp in range(4):
                    gi = nc.scalar.activation(gt[b, hp][:], gt[b, hp][:], Gelu,
                                              bias=ovg_s[:, hp:hp + 1],
                                              scale=1.0)
                    if last_exp[0] is not None:
                        add_dep_helper(gi.ins, last_exp[0].ins, sync=False,
                                       reason="group ACT table sets")
                osb = persist.tile([128, 8, C], BF16, tag=f"osb{b}",
                                   name=f"osb{b}")
                for it in range(8):
                    ops = avp.tile([128, 512], F32, tag="av", name=f"op{b}{it}")
                    for hp in range(4):
                        nc.tensor.matmul(
                            ops[:, 0:C],
                            gt[b, hp][:, it * 128:(it + 1) * 128],
                            wo_s[:, hp, :],
                            start=(hp == 0), stop=(hp == 3),
                        )
                    nc.vector.tensor_tensor(osb[:, it, :], ops[:, 0:C],
                                            bout_s[:], mybir.AluOpType.add)
                    if it % 2 == 1:
                        dma.dma_start(
                            out_d[b, 128 * (it - 1):128 * (it + 1)].rearrange(
                                "(t i) c -> i t c", t=2),
                            osb[:, it - 1:it + 1, :])

    nc.compile()
    return nc


def _host_prep(x, w_q, bn_q, w_k, bn_k, w_v, bn_v, w_out, b_out, bn_out,
               pos_table):
    """Fold BN into weights, build exp-bias table, shard across cores."""
    def fold(bn):
        g, b_, m, v = [np.asarray(a, np.float64) for a in bn]
        s = g / np.sqrt(v + EPS)
        return s, b_ - m * s

    sq, oq = fold(bn_q)
    sk, ok = fold(bn_k)
    sv, ov = fold(bn_v)
    so, oo = fold(bn_out)

    def wtile(w, s, ncols):
        w_eff = (np.asarray(w, np.float64) * s[None, :]).astype(np.float32)
        return np.ascontiguousarray(
            w_eff.reshape(-1, 128, ncols).transpose(1, 0, 2)).astype(NPBF16)

    # [128, ct, C] per q/k -> [dt, 128, qk, ct, 128]
    wqk = np.stack([wtile(w_q, sq, C), wtile(w_k, sk, C)], axis=1)
    wqk = np.ascontiguousarray(
        wqk.reshape(128, 2, 2, 2, 128).transpose(3, 0, 1, 2, 4))
    wv = wtile(w_v, sv, IDV)
    wo = wtile(w_out, so, C)

    oqk_t = np.ascontiguousarray(np.stack(
        [oq.astype(np.float32).reshape(2, 128).T,
         ok.astype(np.float32).reshape(2, 128).T], axis=1))
    ovg_t = np.ascontiguousarray(ov.astype(np.float32).reshape(4, 128).T)
    bout_eff = (np.asarray(b_out, np.float64) * so + oo).astype(np.float32)
    bout_t = np.ascontiguousarray(np.broadcast_to(bout_eff, (128, C)))

    # E[h, jt, j1, i] = exp(bias[j, i, h] / SCALE)
    r = np.arange(32)
    pos = np.stack(np.meshgrid(r, r, indexing="ij"), axis=-1).reshape(-1, 2)
    rel = np.abs(pos[:, None, :] - pos[None, :, :])
    idx = rel[..., 0] * 32 + rel[..., 1]                 # [j, i]
    bias = np.asarray(pos_table, np.float32)[idx]        # [j, i, 8]
    etab = np.exp(bias / SCALE).transpose(2, 0, 1)       # [8, j, i]
    etab = np.ascontiguousarray(
        etab.reshape(HEADS, 8, 128, N)).astype(NPBF16)

    x = np.asarray(x, np.float32).reshape(-1, N, C)      # [B, n, C]
    common = dict(wqk=wqk, wv=wv, wo=wo, oqk=oqk_t, ovg=ovg_t,
                  bout=bout_t, etab=etab)
    in_maps = []
    for c in range(NCORES):
        xl = x[c * BLOC:(c + 1) * BLOC]                  # [2, n, C]
        xtl = xl.transpose(0, 2, 1).reshape(BLOC, 2, 128, N).astype(NPBF16)
        in_maps.append(dict(common, xt=np.ascontiguousarray(xtl)))
    return in_maps


def kernel(**inputs):
    if "nc" not in _CACHE:
        _CACHE["nc"] = _build_nc()
    nc = _CACHE["nc"]
    in_maps = _host_prep(**inputs)
    res = run_bass_kernel_spmd(nc, in_maps, core_ids=list(range(NCORES)),
                               trace=bool(int(os.environ.get("KTRACE", "0"))))
    _CACHE["last_result"] = res
    outs = [res.results[c]["out"].reshape(BLOC, 32, 32, C)
            for c in range(NCORES)]
    return np.concatenate(outs, axis=0).astype(np.float32)


if __name__ == "__main__":
    nc = _build_nc()
    print("build + compile OK")
